# revision 1
# baseline (speedup 1.0000x reference)
"""Trainium2 Bass kernel for nn_CoreferenceResolver (coref UNet + pair decoder).

Sharding: core c handles batch b=c//2 and pair-half h=c%2 (496 of 992 pairs).
The gather/cosine/UNet stages are replicated on the two cores sharing a batch;
the extractor linears and group-bilinear decoder are sharded over pairs.
"""
import os
import sys

for _p in ("/opt/trn_rl_repo",):
    if os.path.isdir(_p) and _p not in sys.path:
        sys.path.insert(0, _p)

import numpy as np

import concourse.bass as bass
import concourse.tile as tile
from concourse import bacc, mybir
from concourse.bass_utils import run_bass_kernel_spmd

f32 = mybir.dt.float32
i16 = mybir.dt.int16
AF = mybir.ActivationFunctionType
OP = mybir.AluOpType
f32r = mybir.dt.float32r
bf16 = mybir.dt.bfloat16


def _f(ap):
    return ap.bitcast(mybir.dt.float32)


def _r(ap):
    """View an fp32 AP as float32r for full-rate PE streaming (N>=256)."""
    return ap.bitcast(f32r)

B, L, D, H = 4, 1024, 768, 12
NE, P = 32, 992
BLOCK = 64
G = D // BLOCK          # 12 groups
OUT_CH = 256
NCORES = 8
NH = P // 2             # 496 pairs per core
KD = D // 128           # 6 chunks of the D dim


def build_nc():
    nc = bacc.Bacc("TRN2", target_bir_lowering=False, debug=False, num_devices=NCORES)

    def inp(name, shape, dt=f32):
        return nc.dram_tensor(name, shape, dt, kind="ExternalInput")

    x_b      = inp("x_b", [L, D])
    ent_idx  = inp("ent_idx", [128, 2], i16)
    ent_mask = inp("ent_mask", [NE, 1])
    iota32   = inp("iota32", [NE, 1])
    ident    = inp("ident", [NE, NE])
    smat     = inp("smat", [128, 2], bf16)
    ones_r   = inp("ones_r", [1, 128], f32r)
    hi_f     = inp("hi_f", [1, NH])
    ti_f     = inp("ti_f", [1, NH])
    pair_idx = inp("pair_idx", [128, NH // 16], i16)

    enc1_w9  = inp("enc1_w9", [1, 9 * 64], f32r);        enc1_bp = inp("enc1_bp", [64, 1])
    enc2_w9  = inp("enc2_w9", [64, 9, 128], f32r);   enc2_bp = inp("enc2_bp", [128, 1])
    bott_w9  = inp("bott_w9", [128, 9, 256], f32r);  bott_bp = inp("bott_bp", [128, 2])
    ag2_wgp  = inp("ag2_wgp", [128, 2, 128], f32r)
    ag2_wxp  = inp("ag2_wxp", [128, 128], f32r)
    ag2_psip = inp("ag2_psip", [128, 1], f32r)
    dec2_w9  = inp("dec2_w9", [128, 3, 9, 128], f32r); dec2_bp = inp("dec2_bp", [128, 1])
    ag1_wgp  = inp("ag1_wgp", [128, 64], f32r)
    ag1_wxp  = inp("ag1_wxp", [64, 64], f32r)
    ag1_psip = inp("ag1_psip", [64, 1], f32r)
    dec1_w9a = inp("dec1_w9a", [128, 9, 64], f32r)
    dec1_w9b = inp("dec1_w9b", [64, 9, 64], f32r);   dec1_bp = inp("dec1_bp", [64, 1])
    fin_wp   = inp("fin_wp", [64, 256], f32r);       fin_bp  = inp("fin_bp", [128, 2])

    W1h = inp("W1h", [128, KD, D], f32r)   # head_w[:768] K-chunked
    W2h = inp("W2h", [128, 2, D], f32r)    # head_w[768:] K-chunked
    W1t = inp("W1t", [128, KD, D], f32r)
    W2t = inp("W2t", [128, 2, D], f32r)
    head_bp = inp("head_bp", [128, KD])
    tail_bp = inp("tail_bp", [128, KD])
    wdec = inp("wdec", [128, G, 128], f32r)   # rows 0:64 == rows 64:128 (host-duplicated)
    dec_bp = inp("dec_bp", [2, 1])

    y = nc.dram_tensor("y", [2, NH], f32, kind="ExternalOutput")

    from contextlib import ExitStack
    with tile.TileContext(nc) as tc, ExitStack() as _ctx:
        sbw = _ctx.enter_context(tc.tile_pool(name="sbw", bufs=1))   # persistent
        sbt = _ctx.enter_context(tc.tile_pool(name="sbt", bufs=3))   # rotating temps
        sws = _ctx.enter_context(tc.tile_pool(name="sws", bufs=4))   # streamed W1 chunks

        # ---------------- load persistent tensors ----------------
        def load(t, shape, dt=f32, name=None, early=False):
            tt = sbw.tile(shape, dt, tag=name or t.name)
            (nc.gpsimd if early else nc.sync).dma_start(tt[:], t[:])
            return tt

        t_eidx  = load(ent_idx, [128, 2], i16, "eidx", early=True)
        t_emask = load(ent_mask, [NE, 1], f32, "emask")
        t_iota  = load(iota32, [NE, 1], f32, "iota")
        t_ident = load(ident, [NE, NE], f32, "ident")
        t_smat  = load(smat, [128, 2], bf16, "smat")
        t_ones  = load(ones_r, [1, 128], f32r, "ones")
        t_hif   = load(hi_f, [1, NH], f32, "hif")
        t_tif   = load(ti_f, [1, NH], f32, "tif")
        pu_cm = tc.tile_pool(name="pu", bufs=3, space="PSUM")
        pu = pu_cm.__enter__()

        # ---------------- entity gather + normalize ----------------
        nrm   = sbw.tile([NE, D], f32, tag="nrm")
        nrmT  = sbw.tile([128, KD, NE], f32, tag="nrmT")
        normc = sbw.tile([NE, 1], f32, tag="normc")
        ent_raw = sbt.tile([128, 1, D], f32, tag="entraw")
        nc.gpsimd.dma_gather(ent_raw[:], x_b[:], t_eidx[:],
                             num_idxs=NE, num_idxs_reg=NE, elem_size=D)
        ent = ent_raw[0:NE, 0, :]
        sq = sbt.tile([NE, D], f32, tag="t")
        nc.vector.tensor_mul(sq[:], ent, ent)
        ss = sbt.tile([NE, 1], f32, tag="ss")
        nc.vector.reduce_sum(ss[:], sq[:], axis=mybir.AxisListType.X)
        nc.scalar.sqrt(normc[:], ss[:])
        nc.vector.tensor_single_scalar(normc[:], normc[:], 1e-13, op=OP.max)
        rinv = sbt.tile([NE, 1], f32, tag="rinv")
        nc.vector.reciprocal(rinv[:], normc[:])
        nc.vector.tensor_tensor(out=rinv[:], in0=rinv[:], in1=t_emask[:], op=OP.mult)
        nc.vector.tensor_scalar(out=nrm[:], in0=ent, scalar1=rinv[:],
                                scalar2=None, op0=OP.mult)
        for k in range(KD):
            p_t = pu.tile([128, NE], f32, tag="pu")
            nc.tensor.transpose(p_t[:], nrm[:, k * 128:(k + 1) * 128], t_ident[:])
            nc.vector.tensor_copy(_r(nrmT[:, k, :]), p_t[:])

        t_pidx  = load(pair_idx, [128, NH // 16], i16, "pidx")

        t_enc1w = load(enc1_w9, [1, 9 * 64], f32r, "enc1w")
        t_enc1b = load(enc1_bp, [64, 1], f32, "enc1b")
        t_enc2w = load(enc2_w9, [64, 9, 128], f32r, "enc2w")
        t_enc2b = load(enc2_bp, [128, 1], f32, "enc2b")
        t_bottw = load(bott_w9, [128, 9, 256], f32r, "bottw")
        t_bottb = load(bott_bp, [128, 2], f32, "bottb")
        t_ag2wg = load(ag2_wgp, [128, 2, 128], f32r, "ag2wg")
        t_ag2wx = load(ag2_wxp, [128, 128], f32r, "ag2wx")
        t_ag2ps = load(ag2_psip, [128, 1], f32r, "ag2ps")
        t_dec2w = load(dec2_w9, [128, 3, 9, 128], f32r, "dec2w")
        t_dec2b = load(dec2_bp, [128, 1], f32, "dec2b")
        t_ag1wg = load(ag1_wgp, [128, 64], f32r, "ag1wg")
        t_ag1wx = load(ag1_wxp, [64, 64], f32r, "ag1wx")
        t_ag1ps = load(ag1_psip, [64, 1], f32r, "ag1ps")
        t_dec1wa = load(dec1_w9a, [128, 9, 64], f32r, "dec1wa")
        t_dec1wb = load(dec1_w9b, [64, 9, 64], f32r, "dec1wb")
        t_dec1b = load(dec1_bp, [64, 1], f32, "dec1b")
        t_finw  = load(fin_wp, [64, 256], f32r, "finw")
        t_finb  = load(fin_bp, [128, 2], f32, "finb")
        t_w2h   = load(W2h, [128, 2, D], f32r, "w2h")
        t_w2t   = load(W2t, [128, 2, D], f32r, "w2t")
        t_hbp   = load(head_bp, [128, KD], f32, "hbp")
        t_tbp   = load(tail_bp, [128, KD], f32, "tbp")
        t_wdec  = load(wdec, [128, G, 128], f32r, "wdec")
        t_decb  = load(dec_bp, [2, 1], f32, "decb")

        # ---------------- persistent intermediates ----------------
        img0  = sbw.tile([1, 34 * 34], f32, tag="img0")
        c1p   = sbw.tile([64, 34 * 34], f32, tag="c1p")
        p1p   = sbw.tile([64, 18 * 18], f32, tag="p1p")
        c2p   = sbw.tile([128, 18 * 18], f32, tag="c2p")
        p2p   = sbw.tile([128, 10 * 10], f32, tag="p2p")
        u2p0  = sbw.tile([128, 18 * 18], f32, tag="u2p0")
        u2p1  = sbw.tile([128, 18 * 18], f32, tag="u2p1")
        att2p = sbw.tile([128, 18 * 18], f32, tag="att2p")
        d2s   = sbw.tile([128, 256], f32, tag="d2s")
        u1p   = sbw.tile([128, 34 * 34], f32, tag="u1p")
        att1p = sbw.tile([64, 34 * 34], f32, tag="att1p")
        d1s   = sbw.tile([64, 1024], f32, tag="d1s")
        amap0 = sbw.tile([128, 1024], f32, tag="amap0")
        amap1 = sbw.tile([128, 1024], f32, tag="amap1")

        ew1   = sbw.tile([NE, D], f32, tag="ew1")
        et1   = sbw.tile([NE, D], f32, tag="et1")
        ohhi  = sbw.tile([NE, NH], f32, tag="ohhi")
        ohti  = sbw.tile([NE, NH], f32, tag="ohti")
        htT0  = sbw.tile([128, NH], f32, tag="htT0")
        htT1  = sbw.tile([128, NH], f32, tag="htT1")
        hsT   = sbw.tile([128, KD, NH], f32, tag="hsT")
        tsT   = sbw.tile([128, KD, NH], f32, tag="tsT")

        # zero the padded borders once (rounded writes: the borders feed f32r matmuls)
        for t in (img0, c1p, p1p, c2p, p2p, u2p0, u2p1, att2p, u1p, att1p):
            nc.gpsimd.memset(t[:], 0.0)

        # ---------------- cosine matrix ----------------
        p_cos = pu.tile([NE, NE], f32, tag="pu")
        for k in range(KD):
            nc.tensor.matmul(p_cos[:], nrmT[:, k, :], nrmT[:, k, :],
                             start=(k == 0), stop=(k == KD - 1))
        s_cos = sbt.tile([NE, NE], f32, tag="scos")
        nc.vector.tensor_copy(_r(s_cos[:]), p_cos[:])

        # ---------------- UNet ----------------
        # enc1: one padded image (DMA issued from DVE right after the cos
        # copy - no cross-queue hop), then 9 taps x 2 halves of K=1 matmuls
        img0v = img0[:].rearrange("c (h w) -> c h w", h=34, w=34)
        nc.gpsimd.dma_start(_r(img0v[0:1, 1:33, 1:33]), _r(s_cos[:]))
        p_c1 = pu.tile([64, 1024], f32, tag="pu")
        for hh in range(2):
            n_mm = 0
            for tap in range(9):
                dy, dx = tap // 3, tap % 3
                rows = slice(dy + 16 * hh, dy + 16 * hh + 16)
                nc.tensor.matmul(p_c1[:, hh * 512:(hh + 1) * 512],
                                 t_enc1w[0:1, tap * 64:(tap + 1) * 64],
                                 _r(img0v[0:1, rows, dx:dx + 32]),
                                 start=(n_mm == 0), stop=(n_mm == 8))
                n_mm += 1
        c1pv = c1p[:].rearrange("c (h w) -> c h w", h=34, w=34)
        for hh in range(2):
            nc.scalar.activation(_r(c1pv[:, 1 + 16 * hh:17 + 16 * hh, 1:33]),
                                 p_c1[:, hh * 512:(hh + 1) * 512].rearrange(
                                     "c (h w) -> c h w", h=16, w=32),
                                 AF.Relu, bias=t_enc1b[:])

        # pool1 -> p1p interior [64, 16, 16]
        p1pv = p1p[:].rearrange("c (h w) -> c h w", h=18, w=18)
        tmp = sbt.tile([64, 16, 16], f32, tag="t")
        nc.vector.tensor_max(tmp[:], c1pv[:, 1:33:2, 1:33:2], c1pv[:, 1:33:2, 2:34:2])
        nc.vector.tensor_max(tmp[:], tmp[:], c1pv[:, 2:34:2, 1:33:2])
        nc.vector.tensor_max(_r(p1pv[:, 1:17, 1:17]), tmp[:], c1pv[:, 2:34:2, 2:34:2])

        # enc2: 9 shifted matmuls K=64
        p_c2 = pu.tile([128, 256], f32, tag="pu")
        for tap in range(9):
            dy, dx = tap // 3, tap % 3
            nc.tensor.matmul(p_c2[:], _r(t_enc2w[:, tap, :]),
                             _r(p1pv[:, dy:dy + 16, dx:dx + 16]),
                             start=(tap == 0), stop=(tap == 8))
        c2pv = c2p[:].rearrange("c (h w) -> c h w", h=18, w=18)
        nc.scalar.activation(_r(c2pv[:, 1:17, 1:17]),
                             p_c2[:].rearrange("c (h w) -> c h w", h=16, w=16),
                             AF.Relu, bias=t_enc2b[:])

        # pool2 -> p2p interior [128, 8, 8]
        p2pv = p2p[:].rearrange("c (h w) -> c h w", h=10, w=10)
        tmp2 = sbt.tile([128, 8, 8], f32, tag="t")
        nc.vector.tensor_max(tmp2[:], c2pv[:, 1:17:2, 1:17:2], c2pv[:, 1:17:2, 2:18:2])
        nc.vector.tensor_max(tmp2[:], tmp2[:], c2pv[:, 2:18:2, 1:17:2])
        nc.vector.tensor_max(_r(p2pv[:, 1:9, 1:9]), tmp2[:], c2pv[:, 2:18:2, 2:18:2])

        # bottleneck: 9 taps x 2 M-chunks, K=128
        c3 = []
        for mc in range(2):
            p_c3 = pu.tile([128, 64], f32, tag="pu")
            for tap in range(9):
                dy, dx = tap // 3, tap % 3
                nc.tensor.matmul(p_c3[:], t_bottw[:, tap, mc * 128:(mc + 1) * 128],
                                 _r(p2pv[:, dy:dy + 8, dx:dx + 8]),
                                 start=(tap == 0), stop=(tap == 8))
            c3s = sbt.tile([128, 8, 8], f32, tag=f"c3_{mc}")
            nc.scalar.activation(c3s[:], p_c3[:].rearrange("c (h w) -> c h w", h=8, w=8),
                                 AF.Relu, bias=t_bottb[:, mc:mc + 1])
            c3.append(c3s)

        # up2 -> u2p interior [128, 16, 16] x2 chunks
        for mc, (src, dst) in enumerate(((c3[0], u2p0), (c3[1], u2p1))):
            dv = dst[:].rearrange("c (h w) -> c h w", h=18, w=18)
            for i in range(2):
                for j in range(2):
                    nc.vector.tensor_copy(_r(dv[:, 1 + i:17:2, 1 + j:17:2]), src[:])

        u2p0v = u2p0[:].rearrange("c (h w) -> c h w", h=18, w=18)
        u2p1v = u2p1[:].rearrange("c (h w) -> c h w", h=18, w=18)

        # attention gate 2: relu(wg@u2 + wx@c2) -> psi -> sigmoid -> c2*a
        p_a2 = pu.tile([128, 256], f32, tag="pu")
        nc.tensor.matmul(p_a2[:], _r(t_ag2wg[:, 0, :]), _r(u2p0v[:, 1:17, 1:17]),
                         start=True, stop=False)
        nc.tensor.matmul(p_a2[:], _r(t_ag2wg[:, 1, :]), _r(u2p1v[:, 1:17, 1:17]),
                         start=False, stop=False)
        nc.tensor.matmul(p_a2[:], _r(t_ag2wx[:]), _r(c2pv[:, 1:17, 1:17]),
                         start=False, stop=True)
        r2 = sbt.tile([128, 256], f32, tag="t")
        nc.scalar.activation(_r(r2[:]), p_a2[:], AF.Relu)
        p_g2 = pu.tile([1, 256], f32, tag="pu")
        nc.tensor.matmul(p_g2[:], t_ag2ps[:], _r(r2[:]))
        a2 = sbt.tile([1, 256], f32, tag="a2")
        nc.scalar.activation(_r(a2[:]), p_g2[:], AF.Sigmoid)
        p_a2b = pu.tile([128, 256], f32, tag="pu")
        nc.tensor.matmul(p_a2b[:], t_ones[:], _r(a2[:]))
        att2pv = att2p[:].rearrange("c (h w) -> c h w", h=18, w=18)
        att2t = sbt.tile([128, 256], f32, tag="t")
        nc.vector.tensor_mul(att2t[:].rearrange("c (h w) -> c h w", h=16, w=16),
                             p_a2b[:].rearrange("c (h w) -> c h w", h=16, w=16),
                             c2pv[:, 1:17, 1:17])
        nc.vector.tensor_copy(_r(att2pv[:, 1:17, 1:17]),
                              att2t[:].rearrange("c (h w) -> c h w", h=16, w=16))

        # dec2: 9 taps x 3 K-chunks (u2p0, u2p1, att2p)
        p_d2 = pu.tile([128, 256], f32, tag="pu")
        srcs2 = (u2p0v, u2p1v, att2pv)
        n_mm = 0
        for tap in range(9):
            dy, dx = tap // 3, tap % 3
            for kc in range(3):
                nc.tensor.matmul(p_d2[:], _r(t_dec2w[:, kc, tap, :]),
                                 _r(srcs2[kc][:, dy:dy + 16, dx:dx + 16]),
                                 start=(n_mm == 0), stop=(n_mm == 26))
                n_mm += 1
        nc.scalar.activation(d2s[:], p_d2[:], AF.Relu, bias=t_dec2b[:])

        # up1 -> u1p interior [128, 32, 32]
        u1pv = u1p[:].rearrange("c (h w) -> c h w", h=34, w=34)
        d2v = d2s[:].rearrange("c (h w) -> c h w", h=16, w=16)
        for i in range(2):
            for j in range(2):
                nc.vector.tensor_copy(_r(u1pv[:, 1 + i:33:2, 1 + j:33:2]), d2v[:])

        # attention gate 1
        p_a1 = pu.tile([64, 1024], f32, tag="pu")
        for hh in range(2):
            rows = slice(1 + 16 * hh, 17 + 16 * hh)
            nc.tensor.matmul(p_a1[:, hh * 512:(hh + 1) * 512], _r(t_ag1wg[:]),
                             _r(u1pv[:, rows, 1:33]), start=True, stop=False)
            nc.tensor.matmul(p_a1[:, hh * 512:(hh + 1) * 512], _r(t_ag1wx[:]),
                             _r(c1pv[:, rows, 1:33]), start=False, stop=True)
        r1 = sbt.tile([64, 1024], f32, tag="t")
        nc.scalar.activation(_r(r1[:]), p_a1[:], AF.Relu)
        p_g1 = pu.tile([1, 1024], f32, tag="pu")
        for hh in range(2):
            nc.tensor.matmul(p_g1[:, hh * 512:(hh + 1) * 512], t_ag1ps[:],
                             _r(r1[:, hh * 512:(hh + 1) * 512]))
        a1 = sbt.tile([1, 1024], f32, tag="a1")
        nc.scalar.activation(_r(a1[:]), p_g1[:], AF.Sigmoid)
        p_a1b = pu.tile([64, 1024], f32, tag="pu")
        for hh in range(2):
            nc.tensor.matmul(p_a1b[:, hh * 512:(hh + 1) * 512], t_ones[:, :64],
                             _r(a1[:, hh * 512:(hh + 1) * 512]))
        att1pv = att1p[:].rearrange("c (h w) -> c h w", h=34, w=34)
        att1t = sbt.tile([64, 1024], f32, tag="t")
        nc.vector.tensor_mul(att1t[:].rearrange("c (h w) -> c h w", h=32, w=32),
                             p_a1b[:].rearrange("c (h w) -> c h w", h=32, w=32),
                             c1pv[:, 1:33, 1:33])
        nc.vector.tensor_copy(_r(att1pv[:, 1:33, 1:33]),
                              att1t[:].rearrange("c (h w) -> c h w", h=32, w=32))

        # dec1: 9 taps x (u1p K=128 + att1p K=64) x 2 N-halves
        p_d1 = pu.tile([64, 1024], f32, tag="pu")
        for hh in range(2):
            n_mm = 0
            for tap in range(9):
                dy, dx = tap // 3, tap % 3
                rows = slice(dy + 16 * hh, dy + 16 * hh + 16)
                nc.tensor.matmul(p_d1[:, hh * 512:(hh + 1) * 512],
                                 _r(t_dec1wa[:, tap, :]), _r(u1pv[:, rows, dx:dx + 32]),
                                 start=(n_mm == 0), stop=False)
                n_mm += 1
                nc.tensor.matmul(p_d1[:, hh * 512:(hh + 1) * 512],
                                 _r(t_dec1wb[:, tap, :]), _r(att1pv[:, rows, dx:dx + 32]),
                                 start=False, stop=(n_mm == 17))
                n_mm += 1
            nc.scalar.activation(_r(d1s[:, hh * 512:(hh + 1) * 512]),
                                 p_d1[:, hh * 512:(hh + 1) * 512],
                                 AF.Relu, bias=t_dec1b[:])

        # fin 1x1 conv -> amapT [256, 1024] in two chunks (with bias, no relu)
        for mc, dst in ((0, amap0), (1, amap1)):
            p_am = pu.tile([128, 1024], f32, tag="pu")
            for hh in range(2):
                nc.tensor.matmul(p_am[:, hh * 512:(hh + 1) * 512],
                                 _r(t_finw[:, mc * 128:(mc + 1) * 128]),
                                 _r(d1s[:, hh * 512:(hh + 1) * 512]))
            nc.scalar.activation(dst[:], p_am[:], AF.Identity, bias=t_finb[:, mc:mc + 1])

        # ---------------- extractor premultiplies ----------------
        # EW1 = ent @ head_w[:768]  (= maxnorm-scaled nrm @ W1), same for tail
        for (wsrc, dst) in ((W1h, ew1), (W1t, et1)):
            p_ew = pu.tile([NE, D], f32, tag="pu")
            for k in range(KD):
                wchunk = sws.tile([128, D], f32r, tag="wbig")
                nc.sync.dma_start(wchunk[:], wsrc[:, k, :])
                for n0, n1 in ((0, 512), (512, 768)):
                    nc.tensor.matmul(p_ew[:, n0:n1],
                                     _r(nrmT[:, k, :]), _r(wchunk[:, n0:n1]),
                                     start=(k == 0), stop=(k == KD - 1))
            nc.scalar.activation(_r(dst[:]), p_ew[:], AF.Copy, scale=normc[:])

        # one-hot selector matrices for hi / ti
        for (src, dst) in ((t_hif, ohhi), (t_tif, ohti)):
            bc = sbt.tile([NE, NH], f32, tag="t")
            nc.gpsimd.partition_broadcast(bc[:], src[:])
            nc.vector.tensor_scalar(out=_r(dst[:]), in0=bc[:], scalar1=t_iota[:],
                                    scalar2=None, op0=OP.is_equal)

        # gather amap columns for each pair: htT = amapT[:, pair_idx]
        htT0x = sbt.tile([128, NH], f32, tag="t")
        htT1x = sbt.tile([128, NH], f32, tag="t")
        nc.gpsimd.ap_gather(htT0x[:].rearrange("c (n o) -> c n o", o=1),
                            amap0[:].rearrange("c (n o) -> c n o", o=1), t_pidx[:],
                            channels=128, num_elems=1024, d=1, num_idxs=NH)
        nc.gpsimd.ap_gather(htT1x[:].rearrange("c (n o) -> c n o", o=1),
                            amap1[:].rearrange("c (n o) -> c n o", o=1), t_pidx[:],
                            channels=128, num_elems=1024, d=1, num_idxs=NH)
        nc.vector.tensor_copy(_r(htT0[:]), htT0x[:])
        nc.vector.tensor_copy(_r(htT1[:]), htT1x[:])

        pu_cm.__exit__(None, None, None)

        # ---------------- pair features + decoder, interleaved per chunk ----
        # for each of the 6 D-chunks: head tanh-arg, tail tanh-arg, then the
        # two decoder groups of that chunk - keeps PE/ACT/DVE pipelined
        ph_cm = tc.tile_pool(name="ph", bufs=4, space="PSUM")
        ph = ph_cm.__enter__()
        pd_cm = tc.tile_pool(name="pd", bufs=2, space="PSUM")
        pd = pd_cm.__enter__()
        po_cm = tc.tile_pool(name="po", bufs=1, space="PSUM")
        po = po_cm.__enter__()
        p_out = po.tile([2, NH], f32, tag="po")
        for k in range(KD):
            cols = slice(k * 128, (k + 1) * 128)
            for (ewt, oh, w2, bp, dstT) in ((ew1, ohhi, t_w2h, t_hbp, hsT),
                                            (et1, ohti, t_w2t, t_tbp, tsT)):
                p_hs = ph.tile([128, NH], f32, tag="ph")
                nc.tensor.matmul(p_hs[:], _r(ewt[:, cols]), _r(oh[:]), start=True, stop=False)
                nc.tensor.matmul(p_hs[:], _r(w2[:, 0, cols]), _r(htT0[:]), start=False, stop=False)
                nc.tensor.matmul(p_hs[:], _r(w2[:, 1, cols]), _r(htT1[:]), start=False, stop=True)
                nc.scalar.activation(_r(dstT[:, k, :]), p_hs[:],
                                     AF.Tanh, bias=bp[:, k:k + 1])
            for half in range(2):
                g = 2 * k + half
                rows = slice(half * 64, (half + 1) * 64)
                p_u = pd.tile([128, NH], f32, tag="pd")
                nc.tensor.matmul(p_u[:], _r(t_wdec[rows, g, :]), _r(tsT[rows, k, :]))
                v = sbt.tile([128, NH], bf16, tag="v")
                nc.vector.tensor_mul(v[0:64, :], p_u[0:64, :], hsT[rows, k, :])
                nc.vector.tensor_mul(v[64:128, :], p_u[64:128, :], hsT[rows, k, :])
                nc.tensor.matmul(p_out[:], t_smat[:], v[:],
                                 start=(g == 0), stop=(g == G - 1))
        out_sb = sbt.tile([2, NH], f32, tag="out")
        nc.scalar.activation(out_sb[:], p_out[:], AF.Identity, bias=t_decb[:])
        nc.sync.dma_start(y[:], out_sb[:])
        po_cm.__exit__(None, None, None)
        pd_cm.__exit__(None, None, None)
        ph_cm.__exit__(None, None, None)

    nc.compile()
    return nc


def f32r_round(a):
    """Round-to-nearest-even to fp32r (11 mantissa bits), matching the PE."""
    u = np.ascontiguousarray(a, np.float32).view(np.uint32).copy()
    u = (u + (np.uint32(0x7FF) + ((u >> np.uint32(12)) & np.uint32(1)))) & np.uint32(0xFFFFF000)
    return u.view(np.float32)


def _wrap16(idx, n_slots):
    """int16 index layout for gpsimd gathers: wrapped in 16 partitions,
    replicated across the 8 gpsimd cores."""
    out = np.zeros((128, n_slots), np.int16)
    for j, v in enumerate(idx):
        out[np.arange(8) * 16 + j % 16, j // 16] = v
    return out


def pack_inputs(inputs):
    """Build the 8 per-core input maps from the full problem inputs."""
    x = np.asarray(inputs["x"], np.float32)
    entity_pos = np.asarray(inputs["entity_pos"])
    hts = np.asarray(inputs["hts"])

    shared = {}
    shared["iota32"] = np.arange(NE, dtype=np.float32).reshape(NE, 1)
    shared["ident"] = np.eye(NE, dtype=np.float32)
    smat = np.zeros((128, 2), np.float32)
    smat[:64, 0] = 1.0
    smat[64:, 1] = 1.0
    shared["smat"] = smat  # cast below
    shared["ones_r"] = np.ones((1, 128), np.float32)

    def W(name):
        return np.asarray(inputs[name], np.float32)

    shared["enc1_w9"] = W("enc1_w").reshape(64, 9).T.reshape(1, 576).copy()
    shared["enc1_bp"] = W("enc1_b").reshape(64, 1)
    shared["enc2_w9"] = W("enc2_w").reshape(128, 64, 9).transpose(1, 2, 0).copy()
    shared["enc2_bp"] = W("enc2_b").reshape(128, 1)
    shared["bott_w9"] = W("bott_w").reshape(256, 128, 9).transpose(1, 2, 0).copy()
    shared["bott_bp"] = W("bott_b").reshape(2, 128).T.copy()
    shared["ag2_wgp"] = W("ag2_wg").reshape(128, 256).T.reshape(2, 128, 128).transpose(1, 0, 2).copy()
    shared["ag2_wxp"] = W("ag2_wx").reshape(128, 128).T.copy()
    shared["ag2_psip"] = W("ag2_psi").reshape(1, 128).T.copy()
    shared["dec2_w9"] = W("dec2_w").reshape(128, 384, 9).transpose(1, 2, 0).reshape(3, 128, 9, 128).transpose(1, 0, 2, 3).copy()
    shared["dec2_bp"] = W("dec2_b").reshape(128, 1)
    shared["ag1_wgp"] = W("ag1_wg").reshape(64, 128).T.copy()
    shared["ag1_wxp"] = W("ag1_wx").reshape(64, 64).T.copy()
    shared["ag1_psip"] = W("ag1_psi").reshape(1, 64).T.copy()
    d1w = W("dec1_w").reshape(64, 192, 9).transpose(1, 2, 0)   # [192, 9, 64]
    shared["dec1_w9a"] = d1w[:128].copy()
    shared["dec1_w9b"] = d1w[128:].copy()
    shared["dec1_bp"] = W("dec1_b").reshape(64, 1)
    shared["fin_wp"] = W("fin_w").reshape(256, 64).T.copy()
    shared["fin_bp"] = W("fin_b").reshape(2, 128).T.copy()

    head_w = W("head_w"); tail_w = W("tail_w")
    shared["W1h"] = head_w[:D].reshape(KD, 128, D).transpose(1, 0, 2).copy()
    shared["W2h"] = head_w[D:].reshape(2, 128, D).transpose(1, 0, 2).copy()
    shared["W1t"] = tail_w[:D].reshape(KD, 128, D).transpose(1, 0, 2).copy()
    shared["W2t"] = tail_w[D:].reshape(2, 128, D).transpose(1, 0, 2).copy()
    shared["head_bp"] = W("head_b").reshape(KD, 128).T.copy()
    shared["tail_bp"] = W("tail_b").reshape(KD, 128).T.copy()
    wd = W("decoder_w").reshape(G, 64, 64, 2).transpose(2, 0, 3, 1).reshape(64, G, 128)
    shared["wdec"] = np.concatenate([wd, wd], axis=0).copy()   # rows duplicated
    shared["dec_bp"] = W("decoder_b").reshape(2, 1)

    for k in ("enc1_w9", "enc2_w9", "bott_w9", "ag2_wgp", "ag2_wxp", "ag2_psip",
              "dec2_w9", "ag1_wgp", "ag1_wxp", "ag1_psip", "dec1_w9a", "dec1_w9b",
              "fin_wp", "W1h", "W2h", "W1t", "W2t", "wdec"):
        shared[k] = f32r_round(shared[k])
    import ml_dtypes
    shared["smat"] = shared["smat"].astype(ml_dtypes.bfloat16)

    in_maps = []
    for c in range(NCORES):
        b, h = c // 2, c % 2
        m = dict(shared)
        m["x_b"] = np.ascontiguousarray(x[b])
        start = entity_pos[b, :, 0].astype(np.int64)
        idx = np.minimum(start + 1, L - 1).astype(np.int16)
        m["ent_idx"] = _wrap16(idx, 2)
        m["ent_mask"] = (start + 1 < L).astype(np.float32).reshape(NE, 1)
        hi = hts[b, h * NH:(h + 1) * NH, 0].astype(np.int64)
        ti = hts[b, h * NH:(h + 1) * NH, 1].astype(np.int64)
        m["hi_f"] = hi.astype(np.float32).reshape(1, NH)
        m["ti_f"] = ti.astype(np.float32).reshape(1, NH)
        m["pair_idx"] = _wrap16((hi * NE + ti).astype(np.int16), NH // 16)
        in_maps.append(m)
    return in_maps


_NC_CACHE = None


def get_nc():
    global _NC_CACHE
    if _NC_CACHE is None:
        _NC_CACHE = build_nc()
    return _NC_CACHE


def kernel(**inputs):
    nc = get_nc()
    in_maps = pack_inputs(inputs)
    res = run_bass_kernel_spmd(nc, in_maps, core_ids=list(range(NCORES)))
    out = np.empty((B * P, 2), np.float32)
    for c in range(NCORES):
        b, h = c // 2, c % 2
        yc = res.results[c]["y"]                  # [2, NH]
        out[b * P + h * NH:b * P + (h + 1) * NH, :] = yc.T
    return out



# revision 8
# speedup vs baseline: 1.2642x; 1.2642x over previous
"""Trainium2 Bass kernel for nn_CoreferenceResolver (coref UNet + pair decoder).

Sharding: core c handles batch b=c//2 and pair-half h=c%2 (496 of 992 pairs).
The gather/cosine/UNet stages are replicated on the two cores sharing a batch;
the extractor linears and group-bilinear decoder are sharded over pairs.

v2 design notes (vs the f32r baseline):
- Host pre-gathers the 32 entity rows (indexing only) and ships them
  transposed (entTb), so the device skips the DRAM gather + PE transposes.
- Cosine matrix via gram trick: gram = entT.T @ entT, norms from the gram
  diagonal, normalization applied with two transpose-by-diag(rinv) PE ops.
- enc1 conv as K=3 im2col: img3 [3, 1090] built with one overlapping-AP DMA.
- All matmul operands bf16 (1.0 PE cycles/row at any N); PSUM stays f32.
- All weights packed into 6 DMAs (vs ~46) to cut HWDGE serialization.
- Decoder inner loop: PE dup-matmul + single [128,496] DVE multiply.
"""
import os
import sys

for _p in ("/opt/trn_rl_repo",):
    if os.path.isdir(_p) and _p not in sys.path:
        sys.path.insert(0, _p)

import numpy as np
import ml_dtypes

import concourse.bass as bass
import concourse.tile as tile
from concourse import bacc, mybir
from concourse.bass_utils import run_bass_kernel_spmd

f32 = mybir.dt.float32
i16 = mybir.dt.int16
bf16 = mybir.dt.bfloat16
AF = mybir.ActivationFunctionType
OP = mybir.AluOpType

B, L, D, H = 4, 1024, 768, 12
NE, P = 32, 992
BLOCK = 64
G = D // BLOCK          # 12 groups
OUT_CH = 256
NCORES = 8
NH = P // 2             # 496 pairs per core
KD = D // 128           # 6 chunks of the D dim

# packS f32 [128, CS] column map
_CS_ENTT = 0      # 96 cols  (bf16 [128, 192])
_CS_IDENT = 96    # 32 cols  (f32 [32, 32])
_CS_IOTA = 128    # 1 col
_CS_PIDX = 129    # 16 cols  (i16 [128, 32])
_CS_SMAT = 145    # 1 col    (bf16 [128, 2])
_CS_DUP = 146     # 64 cols  (bf16 [128, 128])
_CS_E1B = 210
_CS_E2B = 211
_CS_BOB = 212     # 2
_CS_D2B = 214
_CS_D1B = 215
_CS_FIB = 216     # 2
_CS_HBP = 218     # 6
_CS_TBP = 224     # 6
CS = 230

C2 = 1057         # pack2 f32 [2, 1057]: hi 0:496, ti 496:992, ones bf16 992:1056, decb 1056 (all row 0 except decb)

CW1 = 4033        # enc1w3 0:192 | enc2w 192:1344 | bottw 1344:3648 | ag2wg 3648:3904 | ag2wx 3904:4032 | ag2psi 4032
CW2 = 4993        # dec2w 0:3456 | ag1wg 3456:3520 | ag1wx 3520:3584 | ag1psi 3584 | dec1wa 3585:4161 | dec1wb 4161:4737 | finw 4737:4993
CW3 = 9216        # W1h 0:4608 | W1t 4608:9216
CW4 = 4608        # W2h 0:1536 | W2t 1536:3072 | wdec 3072:4608


def build_nc():
    nc = bacc.Bacc("TRN2", target_bir_lowering=False, debug=False, num_devices=NCORES)

    packS = nc.dram_tensor("packS", [128, CS], f32, kind="ExternalInput")
    pack2 = nc.dram_tensor("pack2", [2, C2], f32, kind="ExternalInput")
    pw1 = nc.dram_tensor("pw1", [128, CW1], bf16, kind="ExternalInput")
    pw2 = nc.dram_tensor("pw2", [128, CW2], bf16, kind="ExternalInput")
    pw3 = nc.dram_tensor("pw3", [128, CW3], bf16, kind="ExternalInput")
    pw4 = nc.dram_tensor("pw4", [128, CW4], bf16, kind="ExternalInput")
    y = nc.dram_tensor("y", [2, NH], f32, kind="ExternalOutput")
    DBG = os.environ.get("KDBG") == "1"
    if DBG:
        d_cos = nc.dram_tensor("d_cos", [32, 34], f32, kind="ExternalOutput")
        d_img3 = nc.dram_tensor("d_img3", [3, 1090], f32, kind="ExternalOutput")
        d_c1 = nc.dram_tensor("d_c1", [64, 1156], f32, kind="ExternalOutput")
        d_c2 = nc.dram_tensor("d_c2", [128, 324], f32, kind="ExternalOutput")
        d_d2 = nc.dram_tensor("d_d2", [128, 256], f32, kind="ExternalOutput")
        d_amap0 = nc.dram_tensor("d_amap0", [128, 1024], f32, kind="ExternalOutput")
        d_ew1 = nc.dram_tensor("d_ew1", [32, 768], f32, kind="ExternalOutput")
        d_ohhi = nc.dram_tensor("d_ohhi", [32, NH], f32, kind="ExternalOutput")
        d_htT0 = nc.dram_tensor("d_htT0", [128, NH], f32, kind="ExternalOutput")
        d_hsT = nc.dram_tensor("d_hsT", [128, KD * NH], f32, kind="ExternalOutput")

    from contextlib import ExitStack
    with tile.TileContext(nc) as tc, ExitStack() as _ctx:
        sbw = _ctx.enter_context(tc.tile_pool(name="sbw", bufs=1))   # persistent
        sbt = _ctx.enter_context(tc.tile_pool(name="sbt", bufs=3))   # rotating temps

        # ---------------- persistent tiles ----------------
        tS = sbw.tile([128, CS], f32, tag="tS")
        t2 = sbw.tile([2, C2], f32, tag="t2")
        w1 = sbw.tile([128, CW1], bf16, tag="w1")
        w2 = sbw.tile([128, CW2], bf16, tag="w2")
        w3 = sbw.tile([128, CW3], bf16, tag="w3")
        w4 = sbw.tile([128, CW4], bf16, tag="w4")

        s_cos = sbw.tile([32, 34], bf16, tag="s_cos")
        img0 = sbw.tile([1, 1160], bf16, tag="img0")
        img3 = sbw.tile([3, 1090], bf16, tag="img3")
        c1p = sbw.tile([64, 1156], bf16, tag="c1p")
        p1p = sbw.tile([64, 324], bf16, tag="p1p")
        c2p = sbw.tile([128, 324], bf16, tag="c2p")
        p2p = sbw.tile([128, 100], bf16, tag="p2p")
        u2p0 = sbw.tile([128, 324], bf16, tag="u2p0")
        u2p1 = sbw.tile([128, 324], bf16, tag="u2p1")
        att2p = sbw.tile([128, 324], bf16, tag="att2p")
        d2s = sbw.tile([128, 256], bf16, tag="d2s")
        u1p = sbw.tile([128, 1156], bf16, tag="u1p")
        att1p = sbw.tile([64, 1156], bf16, tag="att1p")
        d1s = sbw.tile([64, 1024], bf16, tag="d1s")
        amap0 = sbw.tile([128, 1024], f32, tag="amap0")
        amap1 = sbw.tile([128, 1024], f32, tag="amap1")
        ew1 = sbw.tile([32, 768], bf16, tag="ew1")
        et1 = sbw.tile([32, 768], bf16, tag="et1")
        ohhi = sbw.tile([32, NH], bf16, tag="ohhi")
        ohti = sbw.tile([32, NH], bf16, tag="ohti")
        htT0f = sbw.tile([128, NH], f32, tag="htT0f")
        htT1f = sbw.tile([128, NH], f32, tag="htT1f")
        htT0 = sbw.tile([128, NH], bf16, tag="htT0")
        htT1 = sbw.tile([128, NH], bf16, tag="htT1")
        hsT = sbw.tile([128, KD, NH], bf16, tag="hsT")
        tsT = sbw.tile([128, KD, NH], bf16, tag="tsT")
        s_gram = sbw.tile([NE, NE], f32, tag="s_gram")
        s_t1 = sbw.tile([NE, NE], f32, tag="s_t1")
        diagR = sbw.tile([NE, NE], f32, tag="diagR")
        normc = sbw.tile([NE, 1], f32, tag="normc")
        rinv = sbw.tile([NE, 1], f32, tag="rinv")
        out_sb = sbw.tile([2, NH], f32, tag="out_sb")

        # ---------------- views into the packs ----------------
        entTb = tS[:, _CS_ENTT:_CS_ENTT + 96].bitcast(bf16).rearrange(
            "p (k e) -> p k e", k=KD)
        identf = tS[0:NE, _CS_IDENT:_CS_IDENT + 32]
        iota = tS[0:NE, _CS_IOTA:_CS_IOTA + 1]
        pidx = tS[:, _CS_PIDX:_CS_PIDX + 16].bitcast(i16)[:, 0:NH // 16]
        smat = tS[:, _CS_SMAT:_CS_SMAT + 1].bitcast(bf16)
        dupm = tS[:, _CS_DUP:_CS_DUP + 64].bitcast(bf16)
        enc1b = tS[0:64, _CS_E1B:_CS_E1B + 1]
        enc2b = tS[:, _CS_E2B:_CS_E2B + 1]
        bottb = tS[:, _CS_BOB:_CS_BOB + 2]
        dec2b = tS[:, _CS_D2B:_CS_D2B + 1]
        dec1b = tS[0:64, _CS_D1B:_CS_D1B + 1]
        finb = tS[:, _CS_FIB:_CS_FIB + 2]
        hbp = tS[:, _CS_HBP:_CS_HBP + 6]
        tbp = tS[:, _CS_TBP:_CS_TBP + 6]

        hi_f = t2[0:1, 0:NH]
        ti_f = t2[0:1, NH:2 * NH]
        onesb = t2[0:1, 992:1056].bitcast(bf16)
        decb = t2[0:2, 1056:1057]

        enc1w = w1[0:3, 0:192]
        enc2w = w1[0:64, 192:1344].rearrange("p (t m) -> p t m", t=9)
        bottw = w1[:, 1344:3648].rearrange("p (t m) -> p t m", t=9)
        ag2wg = w1[:, 3648:3904].rearrange("p (a m) -> p a m", a=2)
        ag2wx = w1[:, 3904:4032]
        ag2psi = w1[:, 4032:4033]

        dec2w = w2[:, 0:3456].rearrange("p (a t m) -> p a t m", a=3, t=9)
        ag1wg = w2[:, 3456:3520]
        ag1wx = w2[0:64, 3520:3584]
        ag1psi = w2[0:64, 3584:3585]
        dec1wa = w2[:, 3585:4161].rearrange("p (t m) -> p t m", t=9)
        dec1wb = w2[0:64, 4161:4737].rearrange("p (t m) -> p t m", t=9)
        finw = w2[0:64, 4737:4993]

        W1h = w3[:, 0:4608].rearrange("p (k m) -> p k m", k=KD)
        W1t = w3[:, 4608:9216].rearrange("p (k m) -> p k m", k=KD)

        W2h = w4[:, 0:1536].rearrange("p (a m) -> p a m", a=2)
        W2t = w4[:, 1536:3072].rearrange("p (a m) -> p a m", a=2)
        wdecv = w4[:, 3072:4608].rearrange("p (g m) -> p g m", g=G)

        # ---------------- Pool: memsets (borders must be zero) -------------
        nc.gpsimd.memset(s_cos[:], 0.0)
        nc.gpsimd.memset(img0[0:1, 0:34], 0.0)
        nc.gpsimd.memset(img0[0:1, 1122:1160], 0.0)
        nc.gpsimd.memset(c1p[:], 0.0)
        nc.gpsimd.memset(p1p[:], 0.0)
        nc.gpsimd.memset(c2p[:], 0.0)
        nc.gpsimd.memset(p2p[:], 0.0)
        nc.gpsimd.memset(u2p0[:], 0.0)
        nc.gpsimd.memset(u2p1[:], 0.0)
        nc.gpsimd.memset(att2p[:], 0.0)
        nc.gpsimd.memset(u1p[:], 0.0)
        nc.gpsimd.memset(att1p[:], 0.0)

        # ---------------- SP: input DMAs (ordering matters) ----------------
        nc.sync.dma_start(tS[:], packS[:])
        nc.sync.dma_start(t2[:], pack2[:])
        nc.sync.dma_start(w1[:], pw1[:])

        pu_cm = tc.tile_pool(name="pu", bufs=2, space="PSUM")
        pu = pu_cm.__enter__()
        pu3_cm = tc.tile_pool(name="pu3", bufs=1, space="PSUM")
        pu3 = pu3_cm.__enter__()

        # ---------------- gram + cosine ----------------
        p_gram = pu.tile([NE, NE], f32, tag="pu")
        for k in range(KD):
            nc.tensor.matmul(p_gram[:], entTb[:, k, :], entTb[:, k, :],
                             start=(k == 0), stop=(k == KD - 1))
        dsq = sbt.tile([NE, NE], f32, tag="t")
        nc.vector.tensor_mul(dsq[:], p_gram[:], identf)
        n2 = sbt.tile([NE, 1], f32, tag="n2")
        nc.vector.reduce_sum(n2[:], dsq[:], axis=mybir.AxisListType.X)
        nc.scalar.sqrt(normc[:], n2[:])
        # dummy sigmoid: hoists the sigmoid/tanh act-table load off the
        # critical path (sqrt lives in a different table set)
        scr = sbt.tile([1, 1], f32, tag="scr")
        nc.scalar.activation(scr[:], normc[0:1, 0:1], AF.Sigmoid)
        nc.vector.tensor_single_scalar(normc[:], normc[:], 1e-13, op=OP.max)
        nc.vector.reciprocal(rinv[:], normc[:])
        # row-scale by rinv, transpose, row-scale again: cos = D gram D
        nc.vector.tensor_scalar(out=s_gram[:], in0=p_gram[:], scalar1=rinv[:],
                                scalar2=None, op0=OP.mult)
        p_t1 = pu.tile([NE, NE], f32, tag="pu")
        nc.tensor.transpose(p_t1[:], s_gram[:], identf)
        nc.vector.tensor_scalar(out=s_cos[:, 1:33], in0=p_t1[:], scalar1=rinv[:],
                                scalar2=None, op0=OP.mult)

        # ---------------- image build: img0 flat, img3 row-shifted ---------
        nc.sync.dma_start(img0[0:1, 34:1122], s_cos[:])
        ap0 = img0[:]
        img3_src = bass.AP(ap0.tensor, ap0.offset, [[1160, 1], [34, 3], [1, 1090]])
        nc.sync.dma_start(img3[:], img3_src)
        nc.sync.dma_start(w2[:], pw2[:])
        nc.sync.dma_start(w3[:], pw3[:])
        nc.sync.dma_start(w4[:], pw4[:])

        # ---------------- one-hot selectors (off critical path) ------------
        for (src, dst) in ((hi_f, ohhi), (ti_f, ohti)):
            bc = sbt.tile([NE, NH], f32, tag="t")
            nc.gpsimd.partition_broadcast(bc[:], src)
            nc.vector.tensor_scalar(out=dst[:], in0=bc[:], scalar1=iota,
                                    scalar2=None, op0=OP.is_equal)

        # ---------------- UNet ----------------
        # enc1: im2col over dy (img3 partitions), dx via base offset; K=3
        p_c1 = pu3.tile([64, 1088], f32, tag="pc1")
        for (w0, wl) in ((0, 512), (512, 512), (1024, 64)):
            for dx in range(3):
                nc.tensor.matmul(p_c1[:, w0:w0 + wl],
                                 enc1w[:, dx * 64:(dx + 1) * 64],
                                 img3[:, dx + w0: dx + w0 + wl],
                                 start=(dx == 0), stop=(dx == 2))
        c1pv = c1p[:].rearrange("c (h w) -> c h w", h=34, w=34)
        nc.scalar.activation(c1pv[:, 1:33, 1:33],
                             p_c1[:].rearrange("c (h w) -> c h w", h=32, w=34)[:, :, 0:32],
                             AF.Relu, bias=enc1b)

        # pool1 -> p1p interior [64, 16, 16]
        p1pv = p1p[:].rearrange("c (h w) -> c h w", h=18, w=18)
        tmp = sbt.tile([64, 16, 16], bf16, tag="t")
        nc.vector.tensor_max(tmp[:], c1pv[:, 1:33:2, 1:33:2], c1pv[:, 1:33:2, 2:34:2])
        nc.vector.tensor_max(tmp[:], tmp[:], c1pv[:, 2:34:2, 1:33:2])
        nc.vector.tensor_max(p1pv[:, 1:17, 1:17], tmp[:], c1pv[:, 2:34:2, 2:34:2])

        # enc2: 9 shifted matmuls K=64
        p_c2 = pu.tile([128, 256], f32, tag="pu")
        for tap in range(9):
            dy, dx = tap // 3, tap % 3
            nc.tensor.matmul(p_c2[:], enc2w[:, tap, :],
                             p1pv[:, dy:dy + 16, dx:dx + 16],
                             start=(tap == 0), stop=(tap == 8))
        c2pv = c2p[:].rearrange("c (h w) -> c h w", h=18, w=18)
        nc.scalar.activation(c2pv[:, 1:17, 1:17],
                             p_c2[:].rearrange("c (h w) -> c h w", h=16, w=16),
                             AF.Relu, bias=enc2b)

        # pool2 -> p2p interior [128, 8, 8]
        p2pv = p2p[:].rearrange("c (h w) -> c h w", h=10, w=10)
        tmp2 = sbt.tile([128, 8, 8], bf16, tag="t")
        nc.vector.tensor_max(tmp2[:], c2pv[:, 1:17:2, 1:17:2], c2pv[:, 1:17:2, 2:18:2])
        nc.vector.tensor_max(tmp2[:], tmp2[:], c2pv[:, 2:18:2, 1:17:2])
        nc.vector.tensor_max(p2pv[:, 1:9, 1:9], tmp2[:], c2pv[:, 2:18:2, 2:18:2])

        # bottleneck: 9 taps x 2 M-chunks, K=128
        c3 = []
        for mc in range(2):
            p_c3 = pu.tile([128, 64], f32, tag="pu")
            for tap in range(9):
                dy, dx = tap // 3, tap % 3
                nc.tensor.matmul(p_c3[:], bottw[:, tap, mc * 128:(mc + 1) * 128],
                                 p2pv[:, dy:dy + 8, dx:dx + 8],
                                 start=(tap == 0), stop=(tap == 8))
            c3s = sbt.tile([128, 8, 8], bf16, tag=f"c3_{mc}")
            nc.scalar.activation(c3s[:], p_c3[:].rearrange("c (h w) -> c h w", h=8, w=8),
                                 AF.Relu, bias=bottb[:, mc:mc + 1])
            c3.append(c3s)

        # up2 -> u2p interior [128, 16, 16] x2 chunks
        for mc, (src, dst) in enumerate(((c3[0], u2p0), (c3[1], u2p1))):
            dv = dst[:].rearrange("c (h w) -> c h w", h=18, w=18)
            for i in range(2):
                for j in range(2):
                    nc.vector.tensor_copy(dv[:, 1 + i:17:2, 1 + j:17:2], src[:])

        u2p0v = u2p0[:].rearrange("c (h w) -> c h w", h=18, w=18)
        u2p1v = u2p1[:].rearrange("c (h w) -> c h w", h=18, w=18)

        # attention gate 2
        p_a2 = pu.tile([128, 256], f32, tag="pu")
        nc.tensor.matmul(p_a2[:], ag2wg[:, 0, :], u2p0v[:, 1:17, 1:17],
                         start=True, stop=False)
        nc.tensor.matmul(p_a2[:], ag2wg[:, 1, :], u2p1v[:, 1:17, 1:17],
                         start=False, stop=False)
        nc.tensor.matmul(p_a2[:], ag2wx[:], c2pv[:, 1:17, 1:17],
                         start=False, stop=True)
        r2 = sbt.tile([128, 256], bf16, tag="t")
        nc.scalar.activation(r2[:], p_a2[:], AF.Relu)
        p_g2 = pu.tile([1, 256], f32, tag="pu")
        nc.tensor.matmul(p_g2[:], ag2psi, r2[:])
        a2 = sbt.tile([1, 256], bf16, tag="a2")
        nc.scalar.activation(a2[:], p_g2[:], AF.Sigmoid)
        p_a2b = pu.tile([128, 256], f32, tag="pu")
        nc.tensor.matmul(p_a2b[:], onesb, a2[:])
        att2pv = att2p[:].rearrange("c (h w) -> c h w", h=18, w=18)
        nc.vector.tensor_mul(att2pv[:, 1:17, 1:17],
                             p_a2b[:].rearrange("c (h w) -> c h w", h=16, w=16),
                             c2pv[:, 1:17, 1:17])

        # dec2: 9 taps x 3 K-chunks (u2p0, u2p1, att2p)
        p_d2 = pu.tile([128, 256], f32, tag="pu")
        srcs2 = (u2p0v, u2p1v, att2pv)
        n_mm = 0
        for tap in range(9):
            dy, dx = tap // 3, tap % 3
            for kc in range(3):
                nc.tensor.matmul(p_d2[:], dec2w[:, kc, tap, :],
                                 srcs2[kc][:, dy:dy + 16, dx:dx + 16],
                                 start=(n_mm == 0), stop=(n_mm == 26))
                n_mm += 1
        nc.scalar.activation(d2s[:], p_d2[:], AF.Relu, bias=dec2b)

        # up1 -> u1p interior [128, 32, 32]
        u1pv = u1p[:].rearrange("c (h w) -> c h w", h=34, w=34)
        d2v = d2s[:].rearrange("c (h w) -> c h w", h=16, w=16)
        for i in range(2):
            for j in range(2):
                nc.vector.tensor_copy(u1pv[:, 1 + i:33:2, 1 + j:33:2], d2v[:])

        # attention gate 1
        p_a1 = pu.tile([64, 1024], f32, tag="pu")
        for hh in range(2):
            rows = slice(1 + 16 * hh, 17 + 16 * hh)
            nc.tensor.matmul(p_a1[:, hh * 512:(hh + 1) * 512], ag1wg[:],
                             u1pv[:, rows, 1:33], start=True, stop=False)
            nc.tensor.matmul(p_a1[:, hh * 512:(hh + 1) * 512], ag1wx[:],
                             c1pv[:, rows, 1:33], start=False, stop=True)
        r1 = sbt.tile([64, 1024], bf16, tag="t")
        nc.scalar.activation(r1[:], p_a1[:], AF.Relu)
        p_g1 = pu.tile([1, 1024], f32, tag="pu")
        for hh in range(2):
            nc.tensor.matmul(p_g1[:, hh * 512:(hh + 1) * 512], ag1psi,
                             r1[:, hh * 512:(hh + 1) * 512])
        a1 = sbt.tile([1, 1024], bf16, tag="a1")
        nc.scalar.activation(a1[:], p_g1[:], AF.Sigmoid)
        p_a1b = pu.tile([64, 1024], f32, tag="pu")
        for hh in range(2):
            nc.tensor.matmul(p_a1b[:, hh * 512:(hh + 1) * 512], onesb[:, 0:64],
                             a1[:, hh * 512:(hh + 1) * 512])
        att1pv = att1p[:].rearrange("c (h w) -> c h w", h=34, w=34)
        nc.vector.tensor_mul(att1pv[:, 1:33, 1:33],
                             p_a1b[:].rearrange("c (h w) -> c h w", h=32, w=32),
                             c1pv[:, 1:33, 1:33])

        # dec1: 9 taps x (u1p K=128 + att1p K=64) x 2 N-halves
        p_d1 = pu.tile([64, 1024], f32, tag="pu")
        for hh in range(2):
            n_mm = 0
            for tap in range(9):
                dy, dx = tap // 3, tap % 3
                rows = slice(dy + 16 * hh, dy + 16 * hh + 16)
                nc.tensor.matmul(p_d1[:, hh * 512:(hh + 1) * 512],
                                 dec1wa[:, tap, :], u1pv[:, rows, dx:dx + 32],
                                 start=(n_mm == 0), stop=False)
                n_mm += 1
                nc.tensor.matmul(p_d1[:, hh * 512:(hh + 1) * 512],
                                 dec1wb[:, tap, :], att1pv[:, rows, dx:dx + 32],
                                 start=False, stop=(n_mm == 17))
                n_mm += 1
            nc.scalar.activation(d1s[:, hh * 512:(hh + 1) * 512],
                                 p_d1[:, hh * 512:(hh + 1) * 512],
                                 AF.Relu, bias=dec1b)

        # fin 1x1 conv -> amapT [256, 1024] in two chunks (bias, no relu)
        for mc, dst in ((0, amap0), (1, amap1)):
            p_am = pu.tile([128, 1024], f32, tag="pu")
            for hh in range(2):
                nc.tensor.matmul(p_am[:, hh * 512:(hh + 1) * 512],
                                 finw[:, mc * 128:(mc + 1) * 128],
                                 d1s[:, hh * 512:(hh + 1) * 512])
            nc.scalar.activation(dst[:], p_am[:], AF.Identity, bias=finb[:, mc:mc + 1])

        # ---------------- extractor premultiplies ----------------
        # EW1 = ent @ head_w[:768] (entTb already unnormalized ent, transposed)
        for (wsrc, dst) in ((W1h, ew1), (W1t, et1)):
            p_ew = pu.tile([NE, D], f32, tag="pu")
            for k in range(KD):
                for n0, n1 in ((0, 512), (512, 768)):
                    nc.tensor.matmul(p_ew[:, n0:n1],
                                     entTb[:, k, :], wsrc[:, k, n0:n1],
                                     start=(k == 0), stop=(k == KD - 1))
            nc.scalar.activation(dst[:], p_ew[:], AF.Identity)

        # gather amap columns for each pair: htT = amapT[:, pair_idx]
        nc.gpsimd.ap_gather(htT0f[:].rearrange("c (n o) -> c n o", o=1),
                            amap0[:].rearrange("c (n o) -> c n o", o=1), pidx,
                            channels=128, num_elems=1024, d=1, num_idxs=NH)
        nc.gpsimd.ap_gather(htT1f[:].rearrange("c (n o) -> c n o", o=1),
                            amap1[:].rearrange("c (n o) -> c n o", o=1), pidx,
                            channels=128, num_elems=1024, d=1, num_idxs=NH)
        nc.vector.tensor_copy(htT0[:], htT0f[:])
        nc.vector.tensor_copy(htT1[:], htT1f[:])

        pu3_cm.__exit__(None, None, None)
        pu_cm.__exit__(None, None, None)

        # ---------------- pair features + decoder, interleaved per chunk ---
        ph_cm = tc.tile_pool(name="ph", bufs=3, space="PSUM")
        ph = ph_cm.__enter__()
        pd_cm = tc.tile_pool(name="pd", bufs=2, space="PSUM")
        pd = pd_cm.__enter__()
        po_cm = tc.tile_pool(name="po", bufs=1, space="PSUM")
        po = po_cm.__enter__()
        p_out = po.tile([2, NH], f32, tag="po")
        for k in range(KD):
            cols = slice(k * 128, (k + 1) * 128)
            for (ewt, oh, w2v, bp, dstT) in ((ew1, ohhi, W2h, hbp, hsT),
                                             (et1, ohti, W2t, tbp, tsT)):
                p_hs = ph.tile([128, NH], f32, tag="ph")
                nc.tensor.matmul(p_hs[:], ewt[:, cols], oh[:], start=True, stop=False)
                nc.tensor.matmul(p_hs[:], w2v[:, 0, cols], htT0[:], start=False, stop=False)
                nc.tensor.matmul(p_hs[:], w2v[:, 1, cols], htT1[:], start=False, stop=True)
                nc.scalar.activation(dstT[:, k, :], p_hs[:],
                                     AF.Tanh, bias=bp[:, k:k + 1])
            for half in range(2):
                g = 2 * k + half
                rows = slice(half * 64, (half + 1) * 64)
                p_u = pd.tile([128, NH], f32, tag="pd")
                nc.tensor.matmul(p_u[:], wdecv[rows, g, :], tsT[rows, k, :])
                v = sbt.tile([128, NH], bf16, tag="v")
                nc.vector.tensor_mul(v[0:64, :], p_u[0:64, :], hsT[rows, k, :])
                nc.vector.tensor_mul(v[64:128, :], p_u[64:128, :], hsT[rows, k, :])
                nc.tensor.matmul(p_out[:], smat, v[:],
                                 start=(g == 0), stop=(g == G - 1))
        nc.scalar.activation(out_sb[:], p_out[:], AF.Identity, bias=decb)
        nc.sync.dma_start(y[:], out_sb[:])
        if DBG:
            def dump(dst, src_ap, shape, dt=bf16):
                tmpd = sbw.tile(shape, f32, tag="dbg_" + dst.name)
                nc.vector.tensor_copy(tmpd[:], src_ap)
                nc.sync.dma_start(dst[:], tmpd[:])
            dump(d_cos, s_cos[:], [32, 34])
            dump(d_img3, img3[:], [3, 1090])
            dump(d_c1, c1p[:], [64, 1156])
            dump(d_c2, c2p[:], [128, 324])
            dump(d_d2, d2s[:], [128, 256])
            nc.sync.dma_start(d_amap0[:], amap0[:])
            dump(d_ew1, ew1[:], [32, 768])
            dump(d_ohhi, ohhi[:], [32, NH])
            nc.sync.dma_start(d_htT0[:], htT0f[:])
            dump(d_hsT, hsT[:].rearrange("p a b -> p (a b)"), [128, KD * NH])
        po_cm.__exit__(None, None, None)
        pd_cm.__exit__(None, None, None)
        ph_cm.__exit__(None, None, None)

    nc.compile()
    return nc


def _wrap16(idx, n_slots):
    """int16 index layout for gpsimd gathers: wrapped in 16 partitions,
    replicated across the 8 gpsimd cores."""
    out = np.zeros((128, n_slots), np.int16)
    for j, v in enumerate(idx):
        out[np.arange(8) * 16 + j % 16, j // 16] = v
    return out


def pack_inputs(inputs):
    """Build the 8 per-core input maps from the full problem inputs."""
    x = np.asarray(inputs["x"], np.float32)
    entity_pos = np.asarray(inputs["entity_pos"])
    hts = np.asarray(inputs["hts"])

    def W(name):
        return np.asarray(inputs[name], np.float32)

    def b16(a):
        return np.ascontiguousarray(a, np.float32).astype(ml_dtypes.bfloat16)

    # ---- packS shared columns (weights/biases identical across cores) ----
    packS_base = np.zeros((128, CS), np.float32)

    def put_f32(col, a):
        a = np.asarray(a, np.float32)
        packS_base[:a.shape[0], col:col + a.shape[1]] = a

    def put_bf16(col, a):
        v = b16(a).view(np.uint16)
        p, c = v.shape
        buf = np.zeros((p, ((c + 1) // 2) * 2), np.uint16)
        buf[:, :c] = v
        packS_base[:p, col:col + buf.shape[1] // 2] = buf.view(np.float32)

    put_f32(_CS_IDENT, np.eye(NE, dtype=np.float32))
    put_f32(_CS_IOTA, np.arange(NE, dtype=np.float32).reshape(NE, 1))
    smat = np.zeros((128, 2), np.float32)
    smat[:64, 0] = 1.0
    smat[64:, 1] = 1.0
    put_bf16(_CS_SMAT, smat)
    dup = np.zeros((128, 128), np.float32)
    for r in range(128):
        for m in range(128):
            if r % 64 == m % 64:
                dup[r, m] = 1.0
    put_bf16(_CS_DUP, dup)
    put_f32(_CS_E1B, W("enc1_b").reshape(64, 1))
    put_f32(_CS_E2B, W("enc2_b").reshape(128, 1))
    put_f32(_CS_BOB, W("bott_b").reshape(2, 128).T)
    put_f32(_CS_D2B, W("dec2_b").reshape(128, 1))
    put_f32(_CS_D1B, W("dec1_b").reshape(64, 1))
    put_f32(_CS_FIB, W("fin_b").reshape(2, 128).T)
    put_f32(_CS_HBP, W("head_b").reshape(KD, 128).T)
    put_f32(_CS_TBP, W("tail_b").reshape(KD, 128).T)

    # ---- weight packs (shared) ----
    def pack_bf16(total, parts):
        buf = np.zeros((128, total), ml_dtypes.bfloat16)
        for col, a in parts:
            v = b16(a)
            buf[:v.shape[0], col:col + v.shape[1]] = v
        return buf

    enc1w3 = W("enc1_w").reshape(64, 3, 3).transpose(1, 2, 0).reshape(3, 192)
    enc2w = W("enc2_w").reshape(128, 64, 9).transpose(1, 2, 0).reshape(64, 1152)
    bottw = W("bott_w").reshape(256, 128, 9).transpose(1, 2, 0).reshape(128, 2304)
    ag2wg = W("ag2_wg").reshape(128, 256).T.reshape(2, 128, 128).transpose(1, 0, 2).reshape(128, 256)
    ag2wx = W("ag2_wx").reshape(128, 128).T
    ag2psi = W("ag2_psi").reshape(1, 128).T
    pw1 = pack_bf16(CW1, [(0, enc1w3), (192, enc2w), (1344, bottw),
                          (3648, ag2wg), (3904, ag2wx), (4032, ag2psi)])

    dec2w = W("dec2_w").reshape(128, 384, 9).transpose(1, 2, 0).reshape(3, 128, 9, 128).transpose(1, 0, 2, 3).reshape(128, 3456)
    ag1wg = W("ag1_wg").reshape(64, 128).T
    ag1wx = W("ag1_wx").reshape(64, 64).T
    ag1psi = W("ag1_psi").reshape(1, 64).T
    d1w = W("dec1_w").reshape(64, 192, 9).transpose(1, 2, 0)   # [192, 9, 64]
    finw = W("fin_w").reshape(256, 64).T
    pw2 = pack_bf16(CW2, [(0, dec2w), (3456, ag1wg), (3520, ag1wx),
                          (3584, ag1psi), (3585, d1w[:128].reshape(128, 576)),
                          (4161, d1w[128:].reshape(64, 576)), (4737, finw)])

    head_w = W("head_w")
    tail_w = W("tail_w")
    W1h = head_w[:D].reshape(KD, 128, D).transpose(1, 0, 2).reshape(128, 4608)
    W1t = tail_w[:D].reshape(KD, 128, D).transpose(1, 0, 2).reshape(128, 4608)
    pw3 = pack_bf16(CW3, [(0, W1h), (4608, W1t)])

    W2h = head_w[D:].reshape(2, 128, D).transpose(1, 0, 2).reshape(128, 1536)
    W2t = tail_w[D:].reshape(2, 128, D).transpose(1, 0, 2).reshape(128, 1536)
    wd = W("decoder_w").reshape(G, 64, 64, 2).transpose(2, 0, 3, 1).reshape(64, G * 128)
    wdec = np.concatenate([wd, wd], axis=0)
    pw4 = pack_bf16(CW4, [(0, W2h), (1536, W2t), (3072, wdec)])

    in_maps = []
    for c in range(NCORES):
        b, h = c // 2, c % 2
        packS = packS_base.copy()
        start = entity_pos[b, :, 0].astype(np.int64)
        idx = np.minimum(start + 1, L - 1)
        ent = x[b][idx].copy()
        ent[~(start + 1 < L)] = 0.0
        entT = ent.T.reshape(KD, 128, NE).transpose(1, 0, 2).reshape(128, KD * NE)
        v = b16(entT).view(np.uint16)
        packS[:, _CS_ENTT:_CS_ENTT + 96] = v.view(np.float32)
        hi = hts[b, h * NH:(h + 1) * NH, 0].astype(np.int64)
        ti = hts[b, h * NH:(h + 1) * NH, 1].astype(np.int64)
        pidxw = _wrap16((hi * NE + ti).astype(np.int16), NH // 16)
        buf = np.zeros((128, 32), np.int16)
        buf[:, :31] = pidxw
        packS[:, _CS_PIDX:_CS_PIDX + 16] = buf.view(np.float32)

        pack2 = np.zeros((2, C2), np.float32)
        pack2[0, 0:NH] = hi.astype(np.float32)
        pack2[0, NH:2 * NH] = ti.astype(np.float32)
        ones = np.ones((1, 128), ml_dtypes.bfloat16).view(np.uint16)
        pack2[0, 992:1056] = ones.view(np.float32)
        pack2[0, 1056] = W("decoder_b")[0]
        pack2[1, 1056] = W("decoder_b")[1]

        in_maps.append({"packS": packS, "pack2": pack2,
                        "pw1": pw1, "pw2": pw2, "pw3": pw3, "pw4": pw4})
    return in_maps


_NC_CACHE = None


def get_nc():
    global _NC_CACHE
    if _NC_CACHE is None:
        _NC_CACHE = build_nc()
    return _NC_CACHE


def kernel(**inputs):
    nc = get_nc()
    in_maps = pack_inputs(inputs)
    res = run_bass_kernel_spmd(nc, in_maps, core_ids=list(range(NCORES)))
    out = np.empty((B * P, 2), np.float32)
    for c in range(NCORES):
        b, h = c // 2, c % 2
        yc = res.results[c]["y"]                  # [2, NH]
        out[b * P + h * NH:b * P + (h + 1) * NH, :] = yc.T
    return out


# revision 13
# speedup vs baseline: 1.5002x; 1.1867x over previous
"""Trainium2 Bass kernel for nn_CoreferenceResolver (coref UNet + pair decoder).

Sharding: core c handles batch b=c//2 and pair-half h=c%2 (496 of 992 pairs).
The gather/cosine/UNet stages are replicated on the two cores sharing a batch;
the extractor linears and group-bilinear decoder are sharded over pairs.

v2 design notes (vs the f32r baseline):
- Host pre-gathers the 32 entity rows (indexing only) and ships them
  transposed (entTb), so the device skips the DRAM gather + PE transposes.
- Cosine matrix via gram trick: gram = entT.T @ entT, norms from the gram
  diagonal, normalization applied with two transpose-by-diag(rinv) PE ops.
- enc1 conv as K=3 im2col: img3 [3, 1090] built with one overlapping-AP DMA.
- All matmul operands bf16 (1.0 PE cycles/row at any N); PSUM stays f32.
- All weights packed into 6 DMAs (vs ~46) to cut HWDGE serialization.
- Decoder inner loop: PE dup-matmul + single [128,496] DVE multiply.
"""
import os
import sys

for _p in ("/opt/trn_rl_repo",):
    if os.path.isdir(_p) and _p not in sys.path:
        sys.path.insert(0, _p)

import numpy as np
import ml_dtypes

import concourse.bass as bass
import concourse.tile as tile
from concourse import bacc, mybir
from concourse.bass_utils import run_bass_kernel_spmd

f32 = mybir.dt.float32
i16 = mybir.dt.int16
bf16 = mybir.dt.bfloat16
AF = mybir.ActivationFunctionType
OP = mybir.AluOpType

B, L, D, H = 4, 1024, 768, 12
NE, P = 32, 992
BLOCK = 64
G = D // BLOCK          # 12 groups
OUT_CH = 256
NCORES = 8
NH = P // 2             # 496 pairs per core
KD = D // 128           # 6 chunks of the D dim

# packS f32 [128, CS] column map
_CS_ENTT = 0      # 96 cols  (bf16 [128, 192])
_CS_IDENT = 96    # 32 cols  (f32 [32, 32])
_CS_IOTA = 128    # 1 col
_CS_PIDX = 129    # 16 cols  (i16 [128, 32])
_CS_SMAT = 145    # 1 col    (bf16 [128, 2])
_CS_DUP = 146     # 64 cols  (bf16 [128, 128])
_CS_E1B = 210
_CS_E2B = 211
_CS_BOB = 212     # 2
_CS_D2B = 214
_CS_D1B = 215
_CS_FIB = 216     # 2
_CS_HBP = 218     # 6
_CS_TBP = 224     # 6
_CS_MAGIC = 230   # 1 col (int32 0x5f3759df)
CS = 231

C2 = 1057         # pack2 f32 [2, 1057]: hi 0:496, ti 496:992, ones bf16 992:1056, decb 1056 (all row 0 except decb)

CW1 = 4033        # enc1w3 0:192 | enc2w 192:1344 | bottw 1344:3648 | ag2wg 3648:3904 | ag2wx 3904:4032 | ag2psi 4032
CW2 = 4993        # dec2w 0:3456 | ag1wg 3456:3520 | ag1wx 3520:3584 | ag1psi 3584 | dec1wa 3585:4161 | dec1wb 4161:4737 | finw 4737:4993
CW3 = 9216        # W1h 0:4608 | W1t 4608:9216
CW4 = 4608        # W2h 0:1536 | W2t 1536:3072 | wdec 3072:4608


def build_nc():
    nc = bacc.Bacc("TRN2", target_bir_lowering=False, debug=False, num_devices=NCORES)

    packS = nc.dram_tensor("packS", [128, CS], f32, kind="ExternalInput")
    pack2 = nc.dram_tensor("pack2", [2, C2], f32, kind="ExternalInput")
    pw1 = nc.dram_tensor("pw1", [128, CW1], bf16, kind="ExternalInput")
    pw2 = nc.dram_tensor("pw2", [128, CW2], bf16, kind="ExternalInput")
    pw3 = nc.dram_tensor("pw3", [128, CW3], bf16, kind="ExternalInput")
    pw4 = nc.dram_tensor("pw4", [128, CW4], bf16, kind="ExternalInput")
    y = nc.dram_tensor("y", [2, NH], f32, kind="ExternalOutput")
    DBG = os.environ.get("KDBG") == "1"
    if DBG:
        d_cos = nc.dram_tensor("d_cos", [32, 34], f32, kind="ExternalOutput")
        d_img3 = nc.dram_tensor("d_img3", [3, 1090], f32, kind="ExternalOutput")
        d_c1 = nc.dram_tensor("d_c1", [64, 1156], f32, kind="ExternalOutput")
        d_c2 = nc.dram_tensor("d_c2", [128, 324], f32, kind="ExternalOutput")
        d_d2 = nc.dram_tensor("d_d2", [128, 256], f32, kind="ExternalOutput")
        d_amap0 = nc.dram_tensor("d_amap0", [128, 1024], f32, kind="ExternalOutput")
        d_ew1 = nc.dram_tensor("d_ew1", [32, 768], f32, kind="ExternalOutput")
        d_ohhi = nc.dram_tensor("d_ohhi", [32, NH], f32, kind="ExternalOutput")
        d_htT0 = nc.dram_tensor("d_htT0", [128, NH], f32, kind="ExternalOutput")
        d_hsT = nc.dram_tensor("d_hsT", [128, KD * NH], f32, kind="ExternalOutput")

    from contextlib import ExitStack
    with tile.TileContext(nc) as tc, ExitStack() as _ctx:
        sbw = _ctx.enter_context(tc.tile_pool(name="sbw", bufs=1))   # persistent
        sbt = _ctx.enter_context(tc.tile_pool(name="sbt", bufs=3))   # rotating temps

        # ---------------- persistent tiles ----------------
        tS = sbw.tile([128, CS], f32, tag="tS")
        t2 = sbw.tile([2, C2], f32, tag="t2")
        w1 = sbw.tile([128, CW1], bf16, tag="w1")
        w2 = sbw.tile([128, CW2], bf16, tag="w2")
        w3 = sbw.tile([128, CW3], bf16, tag="w3")
        w4 = sbw.tile([128, CW4], bf16, tag="w4")

        s_cos = sbw.tile([32, 34], bf16, tag="s_cos")
        img3 = sbw.tile([3, 1124], bf16, tag="img3")
        c1p = sbw.tile([64, 1156], bf16, tag="c1p")
        p1p = sbw.tile([64, 324], bf16, tag="p1p")
        c2p = sbw.tile([128, 324], bf16, tag="c2p")
        p2p = sbw.tile([128, 100], bf16, tag="p2p")
        u2p0 = sbw.tile([128, 324], bf16, tag="u2p0")
        u2p1 = sbw.tile([128, 324], bf16, tag="u2p1")
        att2p = sbw.tile([128, 324], bf16, tag="att2p")
        d2s = sbw.tile([128, 256], bf16, tag="d2s")
        u1p = sbw.tile([128, 1156], bf16, tag="u1p")
        att1p = sbw.tile([64, 1156], bf16, tag="att1p")
        d1s = sbw.tile([64, 1024], bf16, tag="d1s")
        amap0 = sbw.tile([128, 1024], f32, tag="amap0")
        amap1 = sbw.tile([128, 1024], f32, tag="amap1")
        ew1 = sbw.tile([32, 768], bf16, tag="ew1")
        et1 = sbw.tile([32, 768], bf16, tag="et1")
        ohhi = sbw.tile([32, NH], bf16, tag="ohhi")
        ohti = sbw.tile([32, NH], bf16, tag="ohti")
        htT0f = sbw.tile([128, NH], f32, tag="htT0f")
        htT1f = sbw.tile([128, NH], f32, tag="htT1f")
        htT0 = sbw.tile([128, NH], bf16, tag="htT0")
        htT1 = sbw.tile([128, NH], bf16, tag="htT1")
        hsT = sbw.tile([128, KD, NH], bf16, tag="hsT")
        tsT = sbw.tile([128, KD, NH], bf16, tag="tsT")
        s_gram = sbw.tile([NE, NE], f32, tag="s_gram")
        rinv = sbw.tile([NE, 1], f32, tag="rinv")
        out_sb = sbw.tile([2, NH], f32, tag="out_sb")

        # ---------------- views into the packs ----------------
        entTb = tS[:, _CS_ENTT:_CS_ENTT + 96].bitcast(bf16).rearrange(
            "p (k e) -> p k e", k=KD)
        identf = tS[0:NE, _CS_IDENT:_CS_IDENT + 32]
        iota = tS[0:NE, _CS_IOTA:_CS_IOTA + 1]
        pidx = tS[:, _CS_PIDX:_CS_PIDX + 16].bitcast(i16)[:, 0:NH // 16]
        smat = tS[:, _CS_SMAT:_CS_SMAT + 1].bitcast(bf16)
        dupm = tS[:, _CS_DUP:_CS_DUP + 64].bitcast(bf16)
        enc1b = tS[0:64, _CS_E1B:_CS_E1B + 1]
        enc2b = tS[:, _CS_E2B:_CS_E2B + 1]
        bottb = tS[:, _CS_BOB:_CS_BOB + 2]
        dec2b = tS[:, _CS_D2B:_CS_D2B + 1]
        dec1b = tS[0:64, _CS_D1B:_CS_D1B + 1]
        finb = tS[:, _CS_FIB:_CS_FIB + 2]
        hbp = tS[:, _CS_HBP:_CS_HBP + 6]
        tbp = tS[:, _CS_TBP:_CS_TBP + 6]
        magic = tS[0:NE, _CS_MAGIC:_CS_MAGIC + 1]

        hi_f = t2[0:1, 0:NH]
        ti_f = t2[0:1, NH:2 * NH]
        onesb = t2[0:1, 992:1056].bitcast(bf16)
        decb = t2[0:2, 1056:1057]

        enc1w = w1[0:3, 0:192]
        enc2w = w1[0:64, 192:1344].rearrange("p (t m) -> p t m", t=9)
        bottw = w1[:, 1344:3648].rearrange("p (t m) -> p t m", t=9)
        ag2wg = w1[:, 3648:3904].rearrange("p (a m) -> p a m", a=2)
        ag2wx = w1[:, 3904:4032]
        ag2psi = w1[:, 4032:4033]

        dec2w = w2[:, 0:3456].rearrange("p (a t m) -> p a t m", a=3, t=9)
        ag1wg = w2[:, 3456:3520]
        ag1wx = w2[0:64, 3520:3584]
        ag1psi = w2[0:64, 3584:3585]
        dec1wa = w2[:, 3585:4161].rearrange("p (t m) -> p t m", t=9)
        dec1wb = w2[0:64, 4161:4737].rearrange("p (t m) -> p t m", t=9)
        finw = w2[0:64, 4737:4993]

        W1h = w3[:, 0:4608].rearrange("p (k m) -> p k m", k=KD)
        W1t = w3[:, 4608:9216].rearrange("p (k m) -> p k m", k=KD)

        W2h = w4[:, 0:1536].rearrange("p (a m) -> p a m", a=2)
        W2t = w4[:, 1536:3072].rearrange("p (a m) -> p a m", a=2)
        wdecv = w4[:, 3072:4608].rearrange("p (g m) -> p g m", g=G)

        # ---------------- Pool: memsets (borders must be zero) -------------
        nc.gpsimd.memset(s_cos[:], 0.0)
        nc.gpsimd.memset(img3[:], 0.0)
        nc.gpsimd.memset(c1p[:], 0.0)
        nc.gpsimd.memset(p1p[:], 0.0)
        nc.gpsimd.memset(c2p[:], 0.0)
        nc.gpsimd.memset(p2p[:], 0.0)
        nc.gpsimd.memset(u2p0[:], 0.0)
        nc.gpsimd.memset(u2p1[:], 0.0)
        nc.gpsimd.memset(att2p[:], 0.0)
        nc.gpsimd.memset(u1p[:], 0.0)
        nc.gpsimd.memset(att1p[:], 0.0)

        # ---------------- SP: input DMAs (ordering matters) ----------------
        nc.sync.dma_start(tS[:], packS[:])
        nc.sync.dma_start(t2[:], pack2[:])
        nc.sync.dma_start(w1[:], pw1[:])

        pu_cm = tc.tile_pool(name="pu", bufs=2, space="PSUM")
        pu = pu_cm.__enter__()
        pu3_cm = tc.tile_pool(name="pu3", bufs=1, space="PSUM")
        pu3 = pu3_cm.__enter__()

        # ---------------- gram + cosine ----------------
        p_gram = pu.tile([NE, NE], f32, tag="pu")
        for k in range(KD):
            nc.tensor.matmul(p_gram[:], entTb[:, k, :], entTb[:, k, :],
                             start=(k == 0), stop=(k == KD - 1))
        # dummy sigmoid: hoists the sigmoid/tanh act-table load to t~0
        # (s_cos is memset on Pool first, so the read is defined)
        scr = sbt.tile([1, 1], f32, tag="scr")
        nc.scalar.activation(scr[:], s_cos[0:1, 0:1], AF.Sigmoid)
        dsq = sbt.tile([NE, NE], f32, tag="t")
        nc.vector.tensor_mul(dsq[:], p_gram[:], identf)
        n2 = sbt.tile([NE, 1], f32, tag="n2")
        nc.vector.reduce_sum(n2[:], dsq[:], axis=mybir.AxisListType.X)
        # rinv = rsqrt(max(n2, 1e-26)) via bit-trick + 2 Newton steps (DVE
        # only: avoids the ACT sqrt table set entirely)
        nc.vector.tensor_single_scalar(n2[:], n2[:], 1e-26, op=OP.max)
        i32 = mybir.dt.int32
        ish = sbt.tile([NE, 1], f32, tag="ish")
        nc.vector.tensor_single_scalar(ish[:].bitcast(i32), n2[:].bitcast(i32),
                                       1, op=OP.logical_shift_right)
        nc.vector.tensor_tensor(out=rinv[:].bitcast(i32), in0=magic.bitcast(i32),
                                in1=ish[:].bitcast(i32), op=OP.subtract)
        half_d = sbt.tile([NE, 1], f32, tag="hd")
        nc.vector.tensor_single_scalar(half_d[:], n2[:], -0.5, op=OP.mult)
        for _ in range(2):
            yy = sbt.tile([NE, 1], f32, tag="yy")
            nc.vector.tensor_mul(yy[:], rinv[:], rinv[:])
            nc.vector.tensor_mul(yy[:], yy[:], half_d[:])
            nc.vector.tensor_single_scalar(yy[:], yy[:], 1.5, op=OP.add)
            nc.vector.tensor_mul(rinv[:], rinv[:], yy[:])
        # row-scale by rinv, transpose, row-scale again: cos = D gram D
        nc.vector.tensor_scalar(out=s_gram[:], in0=p_gram[:], scalar1=rinv[:],
                                scalar2=None, op0=OP.mult)
        p_t1 = pu.tile([NE, NE], f32, tag="pu")
        nc.tensor.transpose(p_t1[:], s_gram[:], identf)
        nc.vector.tensor_scalar(out=s_cos[:, 1:33], in0=p_t1[:], scalar1=rinv[:],
                                scalar2=None, op0=OP.mult)

        # ---------------- image build: img3 rows = dy-shifted flat windows -
        # img3[dy, i] = imgflat[34*dy + i] where imgflat is the 34x34 padded
        # cos image; s_cos row q = imgflat[34(q+1) : 34(q+2)].
        nc.sync.dma_start(img3[0:1, 34:1122], s_cos[:])
        nc.sync.dma_start(img3[1:2, 0:1088], s_cos[:])
        nc.sync.dma_start(img3[2:3, 0:1054], s_cos[1:32, :])
        nc.sync.dma_start(w2[:], pw2[:])
        nc.sync.dma_start(w3[:], pw3[:])
        nc.sync.dma_start(w4[:], pw4[:])

        # ---------------- UNet ----------------
        # enc1: im2col over dy (img3 partitions), dx via base offset; K=3
        p_c1 = pu3.tile([64, 1088], f32, tag="pc1")
        for (w0, wl) in ((0, 512), (512, 512), (1024, 64)):
            for dx in range(3):
                nc.tensor.matmul(p_c1[:, w0:w0 + wl],
                                 enc1w[:, dx * 64:(dx + 1) * 64],
                                 img3[:, dx + w0: dx + w0 + wl],
                                 start=(dx == 0), stop=(dx == 2))
        c1pv = c1p[:].rearrange("c (h w) -> c h w", h=34, w=34)
        nc.scalar.activation(c1pv[:, 1:33, 1:33],
                             p_c1[:].rearrange("c (h w) -> c h w", h=32, w=34)[:, :, 0:32],
                             AF.Relu, bias=enc1b)

        # pool1 -> p1p interior [64, 16, 16]
        p1pv = p1p[:].rearrange("c (h w) -> c h w", h=18, w=18)
        tmp = sbt.tile([64, 16, 16], bf16, tag="t")
        nc.vector.tensor_max(tmp[:], c1pv[:, 1:33:2, 1:33:2], c1pv[:, 1:33:2, 2:34:2])
        nc.vector.tensor_max(tmp[:], tmp[:], c1pv[:, 2:34:2, 1:33:2])
        nc.vector.tensor_max(p1pv[:, 1:17, 1:17], tmp[:], c1pv[:, 2:34:2, 2:34:2])

        # enc2: 9 shifted matmuls K=64
        p_c2 = pu.tile([128, 256], f32, tag="pu")
        for tap in range(9):
            dy, dx = tap // 3, tap % 3
            nc.tensor.matmul(p_c2[:], enc2w[:, tap, :],
                             p1pv[:, dy:dy + 16, dx:dx + 16],
                             start=(tap == 0), stop=(tap == 8))
        c2pv = c2p[:].rearrange("c (h w) -> c h w", h=18, w=18)
        nc.scalar.activation(c2pv[:, 1:17, 1:17],
                             p_c2[:].rearrange("c (h w) -> c h w", h=16, w=16),
                             AF.Relu, bias=enc2b)

        # pool2 -> p2p interior [128, 8, 8]
        p2pv = p2p[:].rearrange("c (h w) -> c h w", h=10, w=10)
        tmp2 = sbt.tile([128, 8, 8], bf16, tag="t")
        nc.vector.tensor_max(tmp2[:], c2pv[:, 1:17:2, 1:17:2], c2pv[:, 1:17:2, 2:18:2])
        nc.vector.tensor_max(tmp2[:], tmp2[:], c2pv[:, 2:18:2, 1:17:2])
        nc.vector.tensor_max(p2pv[:, 1:9, 1:9], tmp2[:], c2pv[:, 2:18:2, 2:18:2])

        # bottleneck: 9 taps x 2 M-chunks, K=128
        c3 = []
        for mc in range(2):
            p_c3 = pu.tile([128, 64], f32, tag="pu")
            for tap in range(9):
                dy, dx = tap // 3, tap % 3
                nc.tensor.matmul(p_c3[:], bottw[:, tap, mc * 128:(mc + 1) * 128],
                                 p2pv[:, dy:dy + 8, dx:dx + 8],
                                 start=(tap == 0), stop=(tap == 8))
            c3s = sbt.tile([128, 8, 8], bf16, tag=f"c3_{mc}")
            nc.scalar.activation(c3s[:], p_c3[:].rearrange("c (h w) -> c h w", h=8, w=8),
                                 AF.Relu, bias=bottb[:, mc:mc + 1])
            c3.append(c3s)

        # up2 -> u2p interior [128, 16, 16] x2 chunks
        for mc, (src, dst) in enumerate(((c3[0], u2p0), (c3[1], u2p1))):
            dv = dst[:].rearrange("c (h w) -> c h w", h=18, w=18)
            for i in range(2):
                for j in range(2):
                    nc.vector.tensor_copy(dv[:, 1 + i:17:2, 1 + j:17:2], src[:])

        u2p0v = u2p0[:].rearrange("c (h w) -> c h w", h=18, w=18)
        u2p1v = u2p1[:].rearrange("c (h w) -> c h w", h=18, w=18)

        # attention gate 2
        p_a2 = pu.tile([128, 256], f32, tag="pu")
        nc.tensor.matmul(p_a2[:], ag2wg[:, 0, :], u2p0v[:, 1:17, 1:17],
                         start=True, stop=False)
        nc.tensor.matmul(p_a2[:], ag2wg[:, 1, :], u2p1v[:, 1:17, 1:17],
                         start=False, stop=False)
        nc.tensor.matmul(p_a2[:], ag2wx[:], c2pv[:, 1:17, 1:17],
                         start=False, stop=True)
        r2 = sbt.tile([128, 256], bf16, tag="t")
        nc.scalar.activation(r2[:], p_a2[:], AF.Relu)
        p_g2 = pu.tile([1, 256], f32, tag="pu")
        nc.tensor.matmul(p_g2[:], ag2psi, r2[:])
        a2 = sbt.tile([1, 256], bf16, tag="a2")
        nc.scalar.activation(a2[:], p_g2[:], AF.Sigmoid)
        p_a2b = pu.tile([128, 256], f32, tag="pu")
        nc.tensor.matmul(p_a2b[:], onesb, a2[:])
        att2pv = att2p[:].rearrange("c (h w) -> c h w", h=18, w=18)
        nc.vector.tensor_mul(att2pv[:, 1:17, 1:17],
                             p_a2b[:].rearrange("c (h w) -> c h w", h=16, w=16),
                             c2pv[:, 1:17, 1:17])

        # dec2: 9 taps x 3 K-chunks (u2p0, u2p1, att2p)
        p_d2 = pu.tile([128, 256], f32, tag="pu")
        srcs2 = (u2p0v, u2p1v, att2pv)
        n_mm = 0
        for kc in (0, 1, 2):   # att2p chunk last: overlaps the gate chain
            for tap in range(9):
                dy, dx = tap // 3, tap % 3
                nc.tensor.matmul(p_d2[:], dec2w[:, kc, tap, :],
                                 srcs2[kc][:, dy:dy + 16, dx:dx + 16],
                                 start=(n_mm == 0), stop=(n_mm == 26))
                n_mm += 1
        nc.scalar.activation(d2s[:], p_d2[:], AF.Relu, bias=dec2b)

        # up1 -> u1p interior [128, 32, 32]
        u1pv = u1p[:].rearrange("c (h w) -> c h w", h=34, w=34)
        d2v = d2s[:].rearrange("c (h w) -> c h w", h=16, w=16)
        for i in range(2):
            for j in range(2):
                nc.vector.tensor_copy(u1pv[:, 1 + i:33:2, 1 + j:33:2], d2v[:])

        # attention gate 1: two 16-row halves pipelined through PE/ACT/DVE
        att1pv = att1p[:].rearrange("c (h w) -> c h w", h=34, w=34)
        for hh in range(2):
            rows = slice(1 + 16 * hh, 17 + 16 * hh)
            p_a1 = pu.tile([64, 512], f32, tag="pu")
            nc.tensor.matmul(p_a1[:], ag1wg[:], u1pv[:, rows, 1:33],
                             start=True, stop=False)
            nc.tensor.matmul(p_a1[:], ag1wx[:], c1pv[:, rows, 1:33],
                             start=False, stop=True)
            r1 = sbt.tile([64, 512], bf16, tag="t")
            nc.scalar.activation(r1[:], p_a1[:], AF.Relu)
            p_g1 = pu.tile([1, 512], f32, tag="pu")
            nc.tensor.matmul(p_g1[:], ag1psi, r1[:])
            a1 = sbt.tile([1, 512], bf16, tag="a1")
            nc.scalar.activation(a1[:], p_g1[:], AF.Sigmoid)
            p_a1b = pu.tile([64, 512], f32, tag="pu")
            nc.tensor.matmul(p_a1b[:], onesb[:, 0:64], a1[:])
            nc.vector.tensor_mul(att1pv[:, rows, 1:33],
                                 p_a1b[:].rearrange("c (h w) -> c h w", h=16, w=32),
                                 c1pv[:, rows, 1:33])

        # dec1: 9 taps x (u1p K=128 + att1p K=64) x 2 N-halves
        p_d1 = pu.tile([64, 1024], f32, tag="pu")
        for hh in range(2):
            n_mm = 0
            for (wsel, srcv) in ((dec1wa, u1pv), (dec1wb, att1pv)):
                for tap in range(9):
                    dy, dx = tap // 3, tap % 3
                    rows = slice(dy + 16 * hh, dy + 16 * hh + 16)
                    nc.tensor.matmul(p_d1[:, hh * 512:(hh + 1) * 512],
                                     wsel[:, tap, :], srcv[:, rows, dx:dx + 32],
                                     start=(n_mm == 0), stop=(n_mm == 17))
                    n_mm += 1
            nc.scalar.activation(d1s[:, hh * 512:(hh + 1) * 512],
                                 p_d1[:, hh * 512:(hh + 1) * 512],
                                 AF.Relu, bias=dec1b)

        # fin 1x1 conv -> amapT [256, 1024] in two chunks (bias, no relu)
        for mc, dst in ((0, amap0), (1, amap1)):
            p_am = pu.tile([128, 1024], f32, tag="pu")
            for hh in range(2):
                nc.tensor.matmul(p_am[:, hh * 512:(hh + 1) * 512],
                                 finw[:, mc * 128:(mc + 1) * 128],
                                 d1s[:, hh * 512:(hh + 1) * 512])
            nc.scalar.activation(dst[:], p_am[:], AF.Identity, bias=finb[:, mc:mc + 1])

        # ---------------- extractor premultiplies ----------------
        # EW1 = ent @ head_w[:768] (entTb already unnormalized ent, transposed)
        for (wsrc, dst) in ((W1h, ew1), (W1t, et1)):
            p_ew = pu.tile([NE, D], f32, tag="pu")
            for k in range(KD):
                for n0, n1 in ((0, 512), (512, 768)):
                    nc.tensor.matmul(p_ew[:, n0:n1],
                                     entTb[:, k, :], wsrc[:, k, n0:n1],
                                     start=(k == 0), stop=(k == KD - 1))
            nc.scalar.activation(dst[:], p_ew[:], AF.Identity)

        # one-hot selectors (needed only by the pair stage)
        for (srcf, dst) in ((hi_f, ohhi), (ti_f, ohti)):
            bc = sbt.tile([NE, NH], f32, tag="t")
            nc.gpsimd.partition_broadcast(bc[:], srcf)
            nc.vector.tensor_scalar(out=dst[:], in0=bc[:], scalar1=iota,
                                    scalar2=None, op0=OP.is_equal)

        # gather amap columns for each pair: htT = amapT[:, pair_idx]
        nc.gpsimd.ap_gather(htT0f[:].rearrange("c (n o) -> c n o", o=1),
                            amap0[:].rearrange("c (n o) -> c n o", o=1), pidx,
                            channels=128, num_elems=1024, d=1, num_idxs=NH)
        nc.gpsimd.ap_gather(htT1f[:].rearrange("c (n o) -> c n o", o=1),
                            amap1[:].rearrange("c (n o) -> c n o", o=1), pidx,
                            channels=128, num_elems=1024, d=1, num_idxs=NH)
        nc.vector.tensor_copy(htT0[:], htT0f[:])
        nc.vector.tensor_copy(htT1[:], htT1f[:])

        pu3_cm.__exit__(None, None, None)
        pu_cm.__exit__(None, None, None)

        # ---------------- pair features + decoder, interleaved per chunk ---
        ph_cm = tc.tile_pool(name="ph", bufs=3, space="PSUM")
        ph = ph_cm.__enter__()
        pd_cm = tc.tile_pool(name="pd", bufs=2, space="PSUM")
        pd = pd_cm.__enter__()
        po_cm = tc.tile_pool(name="po", bufs=1, space="PSUM")
        po = po_cm.__enter__()
        p_out = po.tile([2, NH], f32, tag="po")
        for k in range(KD):
            cols = slice(k * 128, (k + 1) * 128)
            for (ewt, oh, w2v, bp, dstT) in ((ew1, ohhi, W2h, hbp, hsT),
                                             (et1, ohti, W2t, tbp, tsT)):
                p_hs = ph.tile([128, NH], f32, tag="ph")
                nc.tensor.matmul(p_hs[:], ewt[:, cols], oh[:], start=True, stop=False)
                nc.tensor.matmul(p_hs[:], w2v[:, 0, cols], htT0[:], start=False, stop=False)
                nc.tensor.matmul(p_hs[:], w2v[:, 1, cols], htT1[:], start=False, stop=True)
                nc.scalar.activation(dstT[:, k, :], p_hs[:],
                                     AF.Tanh, bias=bp[:, k:k + 1])
            for half in range(2):
                g = 2 * k + half
                rows = slice(half * 64, (half + 1) * 64)
                p_u = pd.tile([128, NH], f32, tag="pd")
                nc.tensor.matmul(p_u[:], wdecv[rows, g, :], tsT[rows, k, :])
                v = sbt.tile([128, NH], bf16, tag="v")
                if half == 0:
                    nc.vector.tensor_mul(v[0:64, :], p_u[0:64, :], hsT[rows, k, :])
                    nc.vector.tensor_mul(v[64:128, :], p_u[64:128, :], hsT[rows, k, :])
                else:
                    # shift some elementwise load to ACT: the same-base half
                    # runs as a 2x-mode bf16 SBUF multiply on DVE
                    u_sb = sbt.tile([128, NH], bf16, tag="u_sb")
                    nc.scalar.activation(u_sb[64:128, :], p_u[64:128, :], AF.Identity)
                    nc.vector.tensor_mul(v[0:64, :], p_u[0:64, :], hsT[rows, k, :])
                    nc.vector.tensor_mul(v[64:128, :], u_sb[64:128, :], hsT[rows, k, :])
                nc.tensor.matmul(p_out[:], smat, v[:],
                                 start=(g == 0), stop=(g == G - 1))
        nc.scalar.activation(out_sb[:], p_out[:], AF.Identity, bias=decb)
        nc.sync.dma_start(y[:], out_sb[:])
        if DBG:
            def dump(dst, src_ap, shape, dt=bf16):
                tmpd = sbw.tile(shape, f32, tag="dbg_" + dst.name)
                nc.vector.tensor_copy(tmpd[:], src_ap)
                nc.sync.dma_start(dst[:], tmpd[:])
            dump(d_cos, s_cos[:], [32, 34])
            dump(d_img3, img3[:], [3, 1090])
            dump(d_c1, c1p[:], [64, 1156])
            dump(d_c2, c2p[:], [128, 324])
            dump(d_d2, d2s[:], [128, 256])
            nc.sync.dma_start(d_amap0[:], amap0[:])
            dump(d_ew1, ew1[:], [32, 768])
            dump(d_ohhi, ohhi[:], [32, NH])
            nc.sync.dma_start(d_htT0[:], htT0f[:])
            dump(d_hsT, hsT[:].rearrange("p a b -> p (a b)"), [128, KD * NH])
        po_cm.__exit__(None, None, None)
        pd_cm.__exit__(None, None, None)
        ph_cm.__exit__(None, None, None)

    nc.compile()
    return nc


def _wrap16(idx, n_slots):
    """int16 index layout for gpsimd gathers: wrapped in 16 partitions,
    replicated across the 8 gpsimd cores."""
    out = np.zeros((128, n_slots), np.int16)
    for j, v in enumerate(idx):
        out[np.arange(8) * 16 + j % 16, j // 16] = v
    return out


def pack_inputs(inputs):
    """Build the 8 per-core input maps from the full problem inputs."""
    x = np.asarray(inputs["x"], np.float32)
    entity_pos = np.asarray(inputs["entity_pos"])
    hts = np.asarray(inputs["hts"])

    def W(name):
        return np.asarray(inputs[name], np.float32)

    def b16(a):
        return np.ascontiguousarray(a, np.float32).astype(ml_dtypes.bfloat16)

    # ---- packS shared columns (weights/biases identical across cores) ----
    packS_base = np.zeros((128, CS), np.float32)

    def put_f32(col, a):
        a = np.asarray(a, np.float32)
        packS_base[:a.shape[0], col:col + a.shape[1]] = a

    def put_bf16(col, a):
        v = b16(a).view(np.uint16)
        p, c = v.shape
        buf = np.zeros((p, ((c + 1) // 2) * 2), np.uint16)
        buf[:, :c] = v
        packS_base[:p, col:col + buf.shape[1] // 2] = buf.view(np.float32)

    put_f32(_CS_IDENT, np.eye(NE, dtype=np.float32))
    put_f32(_CS_IOTA, np.arange(NE, dtype=np.float32).reshape(NE, 1))
    smat = np.zeros((128, 2), np.float32)
    smat[:64, 0] = 1.0
    smat[64:, 1] = 1.0
    put_bf16(_CS_SMAT, smat)
    dup = np.zeros((128, 128), np.float32)
    for r in range(128):
        for m in range(128):
            if r % 64 == m % 64:
                dup[r, m] = 1.0
    put_bf16(_CS_DUP, dup)
    put_f32(_CS_E1B, W("enc1_b").reshape(64, 1))
    put_f32(_CS_E2B, W("enc2_b").reshape(128, 1))
    put_f32(_CS_BOB, W("bott_b").reshape(2, 128).T)
    put_f32(_CS_D2B, W("dec2_b").reshape(128, 1))
    put_f32(_CS_D1B, W("dec1_b").reshape(64, 1))
    put_f32(_CS_FIB, W("fin_b").reshape(2, 128).T)
    put_f32(_CS_HBP, W("head_b").reshape(KD, 128).T)
    put_f32(_CS_TBP, W("tail_b").reshape(KD, 128).T)
    packS_base[:NE, _CS_MAGIC] = np.full(NE, 0x5F3759DF, np.int32).view(np.float32)

    # ---- weight packs (shared) ----
    def pack_bf16(total, parts):
        buf = np.zeros((128, total), ml_dtypes.bfloat16)
        for col, a in parts:
            v = b16(a)
            buf[:v.shape[0], col:col + v.shape[1]] = v
        return buf

    enc1w3 = W("enc1_w").reshape(64, 3, 3).transpose(1, 2, 0).reshape(3, 192)
    enc2w = W("enc2_w").reshape(128, 64, 9).transpose(1, 2, 0).reshape(64, 1152)
    bottw = W("bott_w").reshape(256, 128, 9).transpose(1, 2, 0).reshape(128, 2304)
    ag2wg = W("ag2_wg").reshape(128, 256).T.reshape(2, 128, 128).transpose(1, 0, 2).reshape(128, 256)
    ag2wx = W("ag2_wx").reshape(128, 128).T
    ag2psi = W("ag2_psi").reshape(1, 128).T
    pw1 = pack_bf16(CW1, [(0, enc1w3), (192, enc2w), (1344, bottw),
                          (3648, ag2wg), (3904, ag2wx), (4032, ag2psi)])

    dec2w = W("dec2_w").reshape(128, 384, 9).transpose(1, 2, 0).reshape(3, 128, 9, 128).transpose(1, 0, 2, 3).reshape(128, 3456)
    ag1wg = W("ag1_wg").reshape(64, 128).T
    ag1wx = W("ag1_wx").reshape(64, 64).T
    ag1psi = W("ag1_psi").reshape(1, 64).T
    d1w = W("dec1_w").reshape(64, 192, 9).transpose(1, 2, 0)   # [192, 9, 64]
    finw = W("fin_w").reshape(256, 64).T
    pw2 = pack_bf16(CW2, [(0, dec2w), (3456, ag1wg), (3520, ag1wx),
                          (3584, ag1psi), (3585, d1w[:128].reshape(128, 576)),
                          (4161, d1w[128:].reshape(64, 576)), (4737, finw)])

    head_w = W("head_w")
    tail_w = W("tail_w")
    W1h = head_w[:D].reshape(KD, 128, D).transpose(1, 0, 2).reshape(128, 4608)
    W1t = tail_w[:D].reshape(KD, 128, D).transpose(1, 0, 2).reshape(128, 4608)
    pw3 = pack_bf16(CW3, [(0, W1h), (4608, W1t)])

    W2h = head_w[D:].reshape(2, 128, D).transpose(1, 0, 2).reshape(128, 1536)
    W2t = tail_w[D:].reshape(2, 128, D).transpose(1, 0, 2).reshape(128, 1536)
    wd = W("decoder_w").reshape(G, 64, 64, 2).transpose(2, 0, 3, 1).reshape(64, G * 128)
    wdec = np.concatenate([wd, wd], axis=0)
    pw4 = pack_bf16(CW4, [(0, W2h), (1536, W2t), (3072, wdec)])

    in_maps = []
    for c in range(NCORES):
        b, h = c // 2, c % 2
        packS = packS_base.copy()
        start = entity_pos[b, :, 0].astype(np.int64)
        idx = np.minimum(start + 1, L - 1)
        ent = x[b][idx].copy()
        ent[~(start + 1 < L)] = 0.0
        entT = ent.T.reshape(KD, 128, NE).transpose(1, 0, 2).reshape(128, KD * NE)
        v = b16(entT).view(np.uint16)
        packS[:, _CS_ENTT:_CS_ENTT + 96] = v.view(np.float32)
        hi = hts[b, h * NH:(h + 1) * NH, 0].astype(np.int64)
        ti = hts[b, h * NH:(h + 1) * NH, 1].astype(np.int64)
        pidxw = _wrap16((hi * NE + ti).astype(np.int16), NH // 16)
        buf = np.zeros((128, 32), np.int16)
        buf[:, :31] = pidxw
        packS[:, _CS_PIDX:_CS_PIDX + 16] = buf.view(np.float32)

        pack2 = np.zeros((2, C2), np.float32)
        pack2[0, 0:NH] = hi.astype(np.float32)
        pack2[0, NH:2 * NH] = ti.astype(np.float32)
        ones = np.ones((1, 128), ml_dtypes.bfloat16).view(np.uint16)
        pack2[0, 992:1056] = ones.view(np.float32)
        pack2[0, 1056] = W("decoder_b")[0]
        pack2[1, 1056] = W("decoder_b")[1]

        in_maps.append({"packS": packS, "pack2": pack2,
                        "pw1": pw1, "pw2": pw2, "pw3": pw3, "pw4": pw4})
    return in_maps


_NC_CACHE = None


def get_nc():
    global _NC_CACHE
    if _NC_CACHE is None:
        _NC_CACHE = build_nc()
    return _NC_CACHE


def kernel(**inputs):
    nc = get_nc()
    in_maps = pack_inputs(inputs)
    res = run_bass_kernel_spmd(nc, in_maps, core_ids=list(range(NCORES)))
    out = np.empty((B * P, 2), np.float32)
    for c in range(NCORES):
        b, h = c // 2, c % 2
        yc = res.results[c]["y"]                  # [2, NH]
        out[b * P + h * NH:b * P + (h + 1) * NH, :] = yc.T
    return out
